# revision 8
# baseline (speedup 1.0000x reference)
"""TRN2 Bass kernel for nn_Encoder (two-phase LSTM over huge batch).

Self-contained: takes the FULL unsharded inputs, shards the batch across
8 NeuronCores (pure data parallel), runs a Bass/Tile kernel per core via
run_bass_kernel_spmd, and reassembles the full outputs.

Device layout (per core, batch B_c = 65536):
  - batch split into 8 passes of 16*512; slice s=0..15 covers 512 columns
    of a pass; SBUF partition p = 8*s + r  <->  (slice s, feature r).
  - gates land in PSUM [128, 4, 512] banks [F, I, O, G] (pytorch gate
    order in the weight rows is i, f, g, o).
  - F/I/O banks: ONE fp8e4m3 DoubleRow matmul each per step: lhsT
    [128, 2, 128], rhs [128, 2, 512] where plane0 = h8 (fp8 copy of the
    hidden state) against block-diagonal W_hh8, plane1 = the x8 tile
    carrying split-precision input rows (x0hi, x0lo, x1hi, x1lo) and two
    ones rows weighted by the split bias (b_hi, b_lo).  Splitting data
    and bias across row pairs cancels their fp8 quantization error; the
    remaining error (fp8 W_hh/W_x weights + fp8 h) measures ~7e-3 l2 on
    the full model - well inside the 2e-2 budget.
  - G bank (tanh gate, most error-sensitive) stays fp16: two matmuls
    (x-part from a 3-step-packed fp16 x-tile with a ones/bias row, then
    h-part) as in the all-fp16 version.
  - This cuts PE work from 8 to 5 matmuls per step-pass; the PE was the
    baseline bottleneck (99% busy at the ~1.1GHz effective clock).
  - ACT: sigmoid over F/I/O in one instr, tanh over G, tanh(c).
  - DVE (fp16): u=F*c, v=I*G, c=u+v, h=O*tanh(c), plus an fp16->fp8
    copy of h into the DoubleRow rhs plane0.
  - 8 passes run as 8 interleaved chains, steps emitted round-robin so
    PSUM slots rotate across chains.
"""

import os
import sys

for _p in ("/opt/trn_rl_repo", "/root/.axon_site/_ro/trn_rl_repo"):
    if os.path.isdir(_p) and _p not in sys.path:
        sys.path.insert(0, _p)
        break

import numpy as np

import concourse.bacc as bacc
import concourse.mybir as mybir
import concourse.tile as tile
from concourse import bass_utils
import bass_rust
from concourse.alu_op_type import AluOpType

F32 = mybir.dt.float32
F16 = mybir.dt.float16
F8 = mybir.dt.float8e4
NP_F8 = bass_utils.ml_dtypes.float8_e4m3
AF = mybir.ActivationFunctionType
MPM = bass_rust.MatmulPerfMode

B = 524288
N_CORES = 8
B_C = B // N_CORES
N = 512
SLICES = 16
PASS = SLICES * N
N_PASS = B_C // PASS
T_OBS, T_PRE, IN, H = 8, 12, 2, 8
T_ALL = T_OBS + T_PRE
XPACK = 3
N_CHUNK_OBS = (T_OBS + XPACK - 1) // XPACK
N_CHUNK_PRE = (T_PRE + XPACK - 1) // XPACK
N_CHAINS = 8
# PSUM bank order: F, I, O, G (sigmoid banks contiguous, tanh last);
# pytorch gate order in the weight rows is i, f, g, o.
BANK_GATE = [1, 0, 3, 2]


# ---------------------------------------------------------------- host prep

def _q8(a):
    return a.astype(NP_F8).astype(np.float32)


def _make_weights(W_in, b_in, W_ih, W_hh, b_ih, b_hh):
    """Weights for one LSTM phase.

    Returns dict with:
      w_dr  [128, 3, 2, 128] fp8: DoubleRow lhsT per F/I/O bank
             (plane0 = block-diag W_hh8, plane1 = x-split + bias rows)
      w_gx  [XPACK, 128, 128] fp16: G-bank x lhsT (bias on ones row)
      w_gh  [128, 128] fp16: G-bank h lhsT
    """
    Wx = (W_ih @ W_in).astype(np.float32)
    bias = (W_ih @ b_in + b_ih + b_hh).astype(np.float32)
    Wx8 = _q8(Wx)
    Wh8 = _q8(W_hh.astype(np.float32))
    b_hi = _q8(bias)
    b_lo = _q8(bias - b_hi)

    w_dr = np.zeros((128, 3, 2, 128), np.float32)
    for b in range(3):
        g = BANK_GATE[b]
        for s in range(16):
            for r in range(H):
                col = 8 * s + r
                w_dr[8 * s: 8 * s + H, b, 0, col] = Wh8[g * H + r, :]
                w_dr[8 * s + 0, b, 1, col] = Wx8[g * H + r, 0]
                w_dr[8 * s + 1, b, 1, col] = Wx8[g * H + r, 0]
                w_dr[8 * s + 2, b, 1, col] = Wx8[g * H + r, 1]
                w_dr[8 * s + 3, b, 1, col] = Wx8[g * H + r, 1]
                w_dr[8 * s + 4, b, 1, col] = b_hi[g * H + r]
                w_dr[8 * s + 5, b, 1, col] = b_lo[g * H + r]

    g = BANK_GATE[3]
    # sigma-trick: bank G holds 2*(pre-activation) so tanh(g) = 2*sig(2g)-1
    # comes out of the same all-sigmoid ACT instruction as F/I/O.
    w_gx = np.zeros((XPACK, 128, 128), np.float32)
    w_gh = np.zeros((128, 128), np.float32)
    for s in range(16):
        for r in range(H):
            col = 8 * s + r
            for tau in range(XPACK):
                for k in range(IN):
                    w_gx[tau, 8 * s + 2 * tau + k, col] = 2.0 * Wx[g * H + r, k]
                w_gx[tau, 8 * s + 6, col] = 2.0 * bias[g * H + r]
            w_gh[8 * s: 8 * s + H, col] = 2.0 * W_hh[g * H + r, :]
    return {
        "w_dr": w_dr.astype(NP_F8),
        "w_gx": w_gx.astype(np.float16),
        "w_gh": w_gh.astype(np.float16),
    }


def _shuffle_state(aT, dtype=np.float16):
    """[8, B_c] -> [N_PASS, 128, N] device layout (p, 8s+r, n)."""
    return np.ascontiguousarray(
        aT.reshape(H, N_PASS, SLICES, N).transpose(1, 2, 0, 3).reshape(
            N_PASS, 128, N).astype(dtype))


def _unshuffle_state(dev):
    """[N_PASS, 128, N] -> [8, B_c]."""
    return dev.reshape(N_PASS, SLICES, H, N).transpose(2, 0, 1, 3).reshape(
        H, B_C)


def _pack_x16(x):
    """[T, 2, B_c] -> [n_chunk, N_PASS, 128, N] fp16 (G bank):
    row 2*tau+k = x[t0+tau][k], row 6 = ones."""
    T = x.shape[0]
    n_chunk = (T + XPACK - 1) // XPACK
    out = np.zeros((n_chunk, N_PASS, SLICES, 8, N), np.float32)
    out[:, :, :, 6, :] = 1.0
    for tau in range(XPACK):
        for k in range(IN):
            for t3 in range(n_chunk):
                t = t3 * XPACK + tau
                if t < T:
                    out[t3, :, :, 2 * tau + k, :] = x[t, k].reshape(
                        N_PASS, SLICES, N)
    return np.ascontiguousarray(
        out.reshape(n_chunk, N_PASS, 128, N).astype(np.float16))


def _pack_x8(x):
    """[T, 2, B_c] -> [T, N_PASS, 128, N] fp8 split-precision rows:
    rows per slice: x0hi, x0lo, x1hi, x1lo, 1, 1, 0, 0."""
    T = x.shape[0]
    out = np.zeros((T, N_PASS, SLICES, 8, N), np.float32)
    for k in range(IN):
        v = x[:, k, :].reshape(T, N_PASS, SLICES, N)
        hi = _q8(v)
        lo = v - hi
        out[:, :, :, 2 * k, :] = hi
        out[:, :, :, 2 * k + 1, :] = lo
    out[:, :, :, 4, :] = 1.0
    out[:, :, :, 5, :] = 1.0
    return np.ascontiguousarray(
        out.reshape(T, N_PASS, 128, N).astype(NP_F8))


def _prep_core_inputs(inputs, lo, hi, weights):
    g = lambda k: np.asarray(inputs[k], np.float32)
    xo = np.ascontiguousarray(g("obs_traj_rel")[:, lo:hi, :].transpose(0, 2, 1))
    xp = np.ascontiguousarray(g("pre_traj_rel")[:, lo:hi, :].transpose(0, 2, 1))
    d = {}
    d["x_obs"] = _pack_x16(xo)
    d["x_pre"] = _pack_x16(xp)
    d["x8_all"] = np.concatenate([_pack_x8(xo), _pack_x8(xp)], axis=0)
    hT0 = np.ascontiguousarray(g("h0")[lo:hi].T)
    d["hT0"] = _shuffle_state(hT0)
    d["hT0_8"] = _shuffle_state(hT0, NP_F8)
    d["cT0"] = _shuffle_state(np.ascontiguousarray(g("c0")[lo:hi].T))
    d["cT0_pre"] = _shuffle_state(np.ascontiguousarray(g("c0_pre")[lo:hi].T))
    d.update(weights)
    return d


# ------------------------------------------------------------- device build

def _build_kernel(tc, outs, ins):
    nc = tc.nc
    state = tc.alloc_tile_pool(name="state", bufs=1)
    psump = tc.alloc_tile_pool(name="psum", bufs=2, space="PSUM")

    wsb = {}
    for key in ("w_dr_obs", "w_dr_pre"):
        w = state.tile([128, 3, 2, 128], F8, name=key + "_sb", tag=key)
        nc.sync.dma_start(w, ins[key])
        wsb[key] = w
    for key in ("w_gx_obs", "w_gx_pre"):
        w = state.tile([128, XPACK, 128], F16, name=key + "_sb", tag=key)
        nc.sync.dma_start(w, ins[key].rearrange("t p m -> p t m"))
        wsb[key] = w
    for key in ("w_gh_obs", "w_gh_pre"):
        w = state.tile([128, 128], F16, name=key + "_sb", tag=key)
        nc.sync.dma_start(w, ins[key])
        wsb[key] = w

    chains = []
    for ci in range(N_CHAINS):
        ch = {}
        for nm in ("h", "u", "v", "c", "tc", "g2"):
            ch[nm] = state.tile([128, N], F16, name=f"{nm}_{ci}",
                                tag=f"{nm}_{ci}")
        ch["hx8"] = state.tile([128, 2, N], F8, name=f"hx8_{ci}",
                               tag=f"hx8_{ci}")
        ch["T"] = state.tile([128, 4, N], F16, name=f"T_{ci}", tag=f"T_{ci}")
        ch["xs"] = [
            state.tile([128, N], F16, name=f"x_{ci}_{xi}", tag=f"x_{ci}_{xi}")
            for xi in range(2)
        ]
        chains.append(ch)
    def step(ch, which, tau, xt, want_h16):
        wdr = wsb[f"w_dr_{which}"]
        wgx, wgh = wsb[f"w_gx_{which}"], wsb[f"w_gh_{which}"]
        ps = psump.tile([128, 4, 512], F32, name="ps", tag="ps")
        for b in range(3):
            nc.tensor.matmul(ps[:, b, :N], wdr[:, b, :, :], ch["hx8"],
                             start=True, stop=True, perf_mode=MPM.DoubleRow)
        out = ps[:, 3, :N]
        nc.tensor.matmul(out, wgx[:, tau, :], xt, start=True, stop=False)
        # G-bank h-part reads the fp8 h plane (mixed fp16xfp8 matmul is
        # exact on the PE; the fp8 rounding of h costs ~1e-3 extra l2).
        nc.tensor.matmul(out, wgh, ch["hx8"][:, 0, :], start=False, stop=True)
        T = ch["T"]
        nc.scalar.activation(T, ps[:, :, :N], AF.Sigmoid)
        nc.vector.tensor_scalar(ch["g2"], T[:, 3, :], 2.0, -1.0,
                                AluOpType.mult, AluOpType.add)   # G = 2s-1
        nc.vector.tensor_mul(ch["u"], T[:, 0, :], ch["c"])       # u = F*c
        nc.vector.tensor_mul(ch["v"], T[:, 1, :], ch["g2"])      # v = I*G
        nc.vector.tensor_add(ch["c"], ch["u"], ch["v"])          # c = u+v
        nc.scalar.activation(ch["tc"], ch["c"], AF.Tanh)         # tanh(c)
        nc.vector.tensor_mul(ch["hx8"][:, 0, :], T[:, 2, :], ch["tc"])
        if want_h16:  # fp16 h only where the output DMA needs it
            nc.vector.tensor_mul(ch["h"], T[:, 2, :], ch["tc"])
    assert N_PASS == N_CHAINS
    for t in range(T_ALL):
        if t < T_OBS:
            which, tt = "obs", t
        else:
            which, tt = "pre", t - T_OBS
        t3, tau = divmod(tt, XPACK)
        for ci in range(N_CHAINS):
            ch, p = chains[ci], ci
            if t == 0:
                nc.sync.dma_start(ch["h"], ins["hT0"][p])
                nc.sync.dma_start(ch["hx8"][:, 0, :], ins["hT0_8"][p])
                nc.sync.dma_start(ch["c"], ins["cT0"][p])
            if t == T_OBS:
                nc.sync.dma_start(outs["hT_obs"][p], ch["h"])
                nc.sync.dma_start(ch["c"], ins["cT0_pre"][p])
            if tau == 0:
                nc.sync.dma_start(ch["xs"][t3 % 2], ins[f"x_{which}"][t3, p])
            nc.sync.dma_start(ch["hx8"][:, 1, :], ins["x8_all"][t, p])
            step(ch, which, tau, ch["xs"][t3 % 2],
                 want_h16=t in (T_OBS - 1, T_ALL - 1))
    for ci in range(N_CHAINS):
        nc.sync.dma_start(outs["hT_pre"][ci], chains[ci]["h"])
    state.release()
    psump.release()


_CACHED = {}


def _get_program():
    if "nc" in _CACHED:
        return _CACHED["nc"]
    nc = bacc.Bacc("TRN2", target_bir_lowering=False, debug=False,
                   enable_asserts=False, num_devices=N_CORES)
    in_specs = {
        "x_obs": ((N_CHUNK_OBS, N_PASS, 128, N), F16),
        "x_pre": ((N_CHUNK_PRE, N_PASS, 128, N), F16),
        "x8_all": ((T_ALL, N_PASS, 128, N), F8),
        "hT0": ((N_PASS, 128, N), F16),
        "hT0_8": ((N_PASS, 128, N), F8),
        "cT0": ((N_PASS, 128, N), F16),
        "cT0_pre": ((N_PASS, 128, N), F16),
        "w_dr_obs": ((128, 3, 2, 128), F8),
        "w_dr_pre": ((128, 3, 2, 128), F8),
        "w_gx_obs": ((XPACK, 128, 128), F16),
        "w_gx_pre": ((XPACK, 128, 128), F16),
        "w_gh_obs": ((128, 128), F16),
        "w_gh_pre": ((128, 128), F16),
    }
    ins = {
        k: nc.dram_tensor(k, list(s), dt, kind="ExternalInput").ap()
        for k, (s, dt) in in_specs.items()
    }
    outs = {
        k: nc.dram_tensor(k, [N_PASS, 128, N], F16, kind="ExternalOutput").ap()
        for k in ("hT_obs", "hT_pre")
    }
    with tile.TileContext(nc) as tc:
        _build_kernel(tc, outs, ins)
    nc.compile()
    _CACHED["nc"] = nc
    return nc


def run(inputs, trace=False, trace_kwargs=None):
    """Run the kernel on 8 cores; returns ((c_out, x_out), BassKernelResults)."""
    nc = _get_program()
    g = lambda k: np.asarray(inputs[k], np.float32)
    wo = _make_weights(g("W_in"), g("b_in"), g("W_ih_obs"),
                       g("W_hh_obs"), g("b_ih_obs"), g("b_hh_obs"))
    wp = _make_weights(g("W_in"), g("b_in"), g("W_ih_pre"),
                       g("W_hh_pre"), g("b_ih_pre"), g("b_hh_pre"))
    weights = {k + "_obs": v for k, v in wo.items()}
    weights.update({k + "_pre": v for k, v in wp.items()})
    in_maps = [
        _prep_core_inputs(inputs, c * B_C, (c + 1) * B_C, weights)
        for c in range(N_CORES)
    ]
    res = bass_utils.run_bass_kernel_spmd(
        nc, in_maps, core_ids=list(range(N_CORES)), trace=trace,
        **(trace_kwargs or {}))
    hT_obs = np.concatenate(
        [_unshuffle_state(res.results[c]["hT_obs"]) for c in range(N_CORES)],
        axis=1)
    hT_pre = np.concatenate(
        [_unshuffle_state(res.results[c]["hT_pre"]) for c in range(N_CORES)],
        axis=1)
    c_out = hT_obs.reshape(B, H).astype(np.float32)
    x_out = hT_pre.reshape(B, H).astype(np.float32)
    return (c_out, x_out), res


def kernel(**inputs):
    (c_out, x_out), _ = run(inputs)
    return c_out, x_out


# revision 10
# speedup vs baseline: 1.0400x; 1.0400x over previous
"""TRN2 Bass kernel for nn_Encoder (two-phase LSTM over huge batch).

Self-contained: takes the FULL unsharded inputs, shards the batch across
8 NeuronCores (pure data parallel), runs a Bass/Tile kernel per core via
run_bass_kernel_spmd, and reassembles the full outputs.

Device layout (per core, batch B_c = 65536):
  - batch split into 8 passes of 16*512; slice s=0..15 covers 512 columns
    of a pass; SBUF partition p = 8*s + r  <->  (slice s, feature r).
  - gates land in PSUM [128, 4, 512] banks [F, I, O, G] (pytorch gate
    order in the weight rows is i, f, g, o).
  - F/I/O banks: ONE fp8e4m3 DoubleRow matmul each per step: lhsT
    [128, 2, 128], rhs [128, 2, 512] where plane0 = h8 (fp8 copy of the
    hidden state) against block-diagonal W_hh8, plane1 = the x8 tile
    carrying split-precision input rows (x0hi, x0lo, x1hi, x1lo) and two
    ones rows weighted by the split bias (b_hi, b_lo).  Splitting data
    and bias across row pairs cancels their fp8 quantization error; the
    remaining error (fp8 W_hh/W_x weights + fp8 h) measures ~7e-3 l2 on
    the full model - well inside the 2e-2 budget.
  - G bank (tanh gate, most error-sensitive) stays fp16: two matmuls
    (x-part from a 3-step-packed fp16 x-tile with a ones/bias row, then
    h-part) as in the all-fp16 version.
  - This cuts PE work from 8 to 5 matmuls per step-pass; the PE was the
    baseline bottleneck (99% busy at the ~1.1GHz effective clock).
  - ACT: sigmoid over F/I/O in one instr, tanh over G, tanh(c).
  - DVE (fp16): u=F*c, v=I*G, c=u+v, h=O*tanh(c), plus an fp16->fp8
    copy of h into the DoubleRow rhs plane0.
  - 8 passes run as 8 interleaved chains, steps emitted round-robin so
    PSUM slots rotate across chains.
"""

import os
import sys

for _p in ("/opt/trn_rl_repo", "/root/.axon_site/_ro/trn_rl_repo"):
    if os.path.isdir(_p) and _p not in sys.path:
        sys.path.insert(0, _p)
        break

import numpy as np

import concourse.bacc as bacc
import concourse.mybir as mybir
import concourse.tile as tile
from concourse import bass_utils
import bass_rust
from concourse.alu_op_type import AluOpType

F32 = mybir.dt.float32
F16 = mybir.dt.float16
F8 = mybir.dt.float8e4
NP_F8 = bass_utils.ml_dtypes.float8_e4m3
AF = mybir.ActivationFunctionType
MPM = bass_rust.MatmulPerfMode

B = 524288
N_CORES = 8
B_C = B // N_CORES
N = 512
SLICES = 16
PASS = SLICES * N
N_PASS = B_C // PASS
T_OBS, T_PRE, IN, H = 8, 12, 2, 8
T_ALL = T_OBS + T_PRE
XPACK = 3
N_CHUNK_OBS = (T_OBS + XPACK - 1) // XPACK
N_CHUNK_PRE = (T_PRE + XPACK - 1) // XPACK
N_CHAINS = 8
# PSUM bank order: F, I, O, G (sigmoid banks contiguous, tanh last);
# pytorch gate order in the weight rows is i, f, g, o.
BANK_GATE = [1, 0, 3, 2]


# ---------------------------------------------------------------- host prep

def _q8(a):
    return a.astype(NP_F8).astype(np.float32)


def _make_weights(W_in, b_in, W_ih, W_hh, b_ih, b_hh):
    """Weights for one LSTM phase.

    Returns dict with:
      w_dr  [128, 3, 2, 128] fp8: DoubleRow lhsT per F/I/O bank
             (plane0 = block-diag W_hh8, plane1 = x-split + bias rows)
      w_gx  [XPACK, 128, 128] fp16: G-bank x lhsT (bias on ones row)
      w_gh  [128, 128] fp16: G-bank h lhsT
    """
    Wx = (W_ih @ W_in).astype(np.float32)
    bias = (W_ih @ b_in + b_ih + b_hh).astype(np.float32)
    Wx8 = _q8(Wx)
    Wh8 = _q8(W_hh.astype(np.float32))
    b_hi = _q8(bias)
    b_lo = _q8(bias - b_hi)

    w_dr = np.zeros((128, 3, 2, 128), np.float32)
    for b in range(3):
        g = BANK_GATE[b]
        for s in range(16):
            for r in range(H):
                col = 8 * s + r
                w_dr[8 * s: 8 * s + H, b, 0, col] = Wh8[g * H + r, :]
                w_dr[8 * s + 0, b, 1, col] = Wx8[g * H + r, 0]
                w_dr[8 * s + 1, b, 1, col] = Wx8[g * H + r, 0]
                w_dr[8 * s + 2, b, 1, col] = Wx8[g * H + r, 1]
                w_dr[8 * s + 3, b, 1, col] = Wx8[g * H + r, 1]
                w_dr[8 * s + 4, b, 1, col] = b_hi[g * H + r]
                w_dr[8 * s + 5, b, 1, col] = b_lo[g * H + r]

    g = BANK_GATE[3]
    # sigma-trick: bank G holds 2*(pre-activation) so tanh(g) = 2*sig(2g)-1
    # comes out of the same all-sigmoid ACT instruction as F/I/O.
    w_gx = np.zeros((XPACK, 128, 128), np.float32)
    w_gh = np.zeros((128, 128), np.float32)
    for s in range(16):
        for r in range(H):
            col = 8 * s + r
            for tau in range(XPACK):
                for k in range(IN):
                    w_gx[tau, 8 * s + 2 * tau + k, col] = 2.0 * Wx[g * H + r, k]
                w_gx[tau, 8 * s + 6, col] = 2.0 * bias[g * H + r]
            w_gh[8 * s: 8 * s + H, col] = 2.0 * W_hh[g * H + r, :]
    return {
        "w_dr": w_dr.astype(NP_F8),
        "w_gx": w_gx.astype(np.float16),
        "w_gh": w_gh.astype(np.float16),
    }


def _shuffle_state(aT, dtype=np.float16):
    """[8, B_c] -> [N_PASS, 128, N] device layout (p, 8s+r, n)."""
    return np.ascontiguousarray(
        aT.reshape(H, N_PASS, SLICES, N).transpose(1, 2, 0, 3).reshape(
            N_PASS, 128, N).astype(dtype))


def _unshuffle_state(dev):
    """[N_PASS, 128, N] -> [8, B_c]."""
    return dev.reshape(N_PASS, SLICES, H, N).transpose(2, 0, 1, 3).reshape(
        H, B_C)


def _pack_x16(x):
    """[T, 2, B_c] -> [n_chunk, N_PASS, 128, N] fp16 (G bank):
    row 2*tau+k = x[t0+tau][k], row 6 = ones."""
    T = x.shape[0]
    n_chunk = (T + XPACK - 1) // XPACK
    out = np.zeros((n_chunk, N_PASS, SLICES, 8, N), np.float32)
    out[:, :, :, 6, :] = 1.0
    for tau in range(XPACK):
        for k in range(IN):
            for t3 in range(n_chunk):
                t = t3 * XPACK + tau
                if t < T:
                    out[t3, :, :, 2 * tau + k, :] = x[t, k].reshape(
                        N_PASS, SLICES, N)
    return np.ascontiguousarray(
        out.reshape(n_chunk, N_PASS, 128, N).astype(np.float16))


def _pack_x8(x):
    """[T, 2, B_c] -> [T, N_PASS, 128, N] fp8 split-precision rows:
    rows per slice: x0hi, x0lo, x1hi, x1lo, 1, 1, 0, 0."""
    T = x.shape[0]
    out = np.zeros((T, N_PASS, SLICES, 8, N), np.float32)
    for k in range(IN):
        v = x[:, k, :].reshape(T, N_PASS, SLICES, N)
        hi = _q8(v)
        lo = v - hi
        out[:, :, :, 2 * k, :] = hi
        out[:, :, :, 2 * k + 1, :] = lo
    out[:, :, :, 4, :] = 1.0
    out[:, :, :, 5, :] = 1.0
    return np.ascontiguousarray(
        out.reshape(T, N_PASS, 128, N).astype(NP_F8))


def _prep_core_inputs(inputs, lo, hi, weights):
    g = lambda k: np.asarray(inputs[k], np.float32)
    xo = np.ascontiguousarray(g("obs_traj_rel")[:, lo:hi, :].transpose(0, 2, 1))
    xp = np.ascontiguousarray(g("pre_traj_rel")[:, lo:hi, :].transpose(0, 2, 1))
    d = {}
    d["x_obs"] = _pack_x16(xo)
    d["x_pre"] = _pack_x16(xp)
    d["x8_all"] = np.concatenate([_pack_x8(xo), _pack_x8(xp)], axis=0)
    hT0 = np.ascontiguousarray(g("h0")[lo:hi].T)
    d["hT0"] = _shuffle_state(hT0)
    d["hT0_8"] = _shuffle_state(hT0, NP_F8)
    d["cT0"] = _shuffle_state(np.ascontiguousarray(g("c0")[lo:hi].T))
    d["cT0_pre"] = _shuffle_state(np.ascontiguousarray(g("c0_pre")[lo:hi].T))
    d.update(weights)
    return d


# ------------------------------------------------------------- device build

def _build_kernel(tc, outs, ins):
    nc = tc.nc
    state = tc.alloc_tile_pool(name="state", bufs=1)
    psump = tc.alloc_tile_pool(name="psum", bufs=2, space="PSUM")

    wsb = {}
    for key in ("w_dr_obs", "w_dr_pre"):
        w = state.tile([128, 3, 2, 128], F8, name=key + "_sb", tag=key)
        nc.sync.dma_start(w, ins[key])
        wsb[key] = w
    for key in ("w_gx_obs", "w_gx_pre"):
        w = state.tile([128, XPACK, 128], F16, name=key + "_sb", tag=key)
        nc.sync.dma_start(w, ins[key].rearrange("t p m -> p t m"))
        wsb[key] = w
    for key in ("w_gh_obs", "w_gh_pre"):
        w = state.tile([128, 128], F16, name=key + "_sb", tag=key)
        nc.sync.dma_start(w, ins[key])
        wsb[key] = w

    cpair, tcpair = [], []
    for k in range(N_CHAINS // 2):
        cpair.append(state.tile([128, 2, N], F16, name=f"cp_{k}", tag=f"cp_{k}"))
        tcpair.append(state.tile([128, 2, N], F16, name=f"tcp_{k}",
                                 tag=f"tcp_{k}"))
    chains = []
    for ci in range(N_CHAINS):
        ch = {}
        for nm in ("h", "u", "v", "g2"):
            ch[nm] = state.tile([128, N], F16, name=f"{nm}_{ci}",
                                tag=f"{nm}_{ci}")
        ch["c"] = cpair[ci // 2][:, ci % 2]
        ch["tc"] = tcpair[ci // 2][:, ci % 2]
        ch["hx8"] = state.tile([128, 2, N], F8, name=f"hx8_{ci}",
                               tag=f"hx8_{ci}")
        ch["T"] = state.tile([128, 4, N], F16, name=f"T_{ci}", tag=f"T_{ci}")
        ch["xs"] = [
            state.tile([128, N], F16, name=f"x_{ci}_{xi}", tag=f"x_{ci}_{xi}")
            for xi in range(2)
        ]
        chains.append(ch)
    def step(ch, which, tau, xt):
        wdr = wsb[f"w_dr_{which}"]
        wgx, wgh = wsb[f"w_gx_{which}"], wsb[f"w_gh_{which}"]
        ps = psump.tile([128, 4, 512], F32, name="ps", tag="ps")
        for b in range(3):
            nc.tensor.matmul(ps[:, b, :N], wdr[:, b, :, :], ch["hx8"],
                             start=True, stop=True, perf_mode=MPM.DoubleRow)
        out = ps[:, 3, :N]
        nc.tensor.matmul(out, wgx[:, tau, :], xt, start=True, stop=False)
        nc.tensor.matmul(out, wgh, ch["hx8"][:, 0, :], start=False, stop=True)
        T = ch["T"]
        nc.scalar.activation(T, ps[:, :, :N], AF.Sigmoid)
        nc.vector.tensor_scalar(ch["g2"], T[:, 3, :], 2.0, -1.0,
                                AluOpType.mult, AluOpType.add)   # G = 2s-1
        nc.vector.tensor_mul(ch["u"], T[:, 0, :], ch["c"])       # u = F*c
        nc.vector.tensor_mul(ch["v"], T[:, 1, :], ch["g2"])      # v = I*G
        nc.vector.tensor_add(ch["c"], ch["u"], ch["v"])          # c = u+v

    def tail(k, want_h16):
        nc.scalar.activation(tcpair[k], cpair[k], AF.Tanh)       # tanh(c) x2
        for ci in (2 * k, 2 * k + 1):
            ch = chains[ci]
            nc.vector.tensor_mul(ch["hx8"][:, 0, :], ch["T"][:, 2, :],
                                 ch["tc"])                       # h8 = O*tc
            if want_h16:
                nc.vector.tensor_mul(ch["h"], ch["T"][:, 2, :], ch["tc"])
    assert N_PASS == N_CHAINS
    pend = []
    for t in range(T_ALL):
        if t < T_OBS:
            which, tt = "obs", t
        else:
            which, tt = "pre", t - T_OBS
        t3, tau = divmod(tt, XPACK)
        want_h16 = t in (T_OBS - 1, T_ALL - 1)
        for ci in range(N_CHAINS):
            ch, p = chains[ci], ci
            if t == 0:
                nc.sync.dma_start(ch["hx8"][:, 0, :], ins["hT0_8"][p])
                nc.sync.dma_start(ch["h"], ins["hT0"][p])
                nc.sync.dma_start(ch["c"], ins["cT0"][p])
            if t == T_OBS:
                nc.sync.dma_start(outs["hT_obs"][p], ch["h"])
                nc.sync.dma_start(ch["c"], ins["cT0_pre"][p])
            if tau == 0:
                nc.sync.dma_start(ch["xs"][t3 % 2], ins[f"x_{which}"][t3, p])
            nc.sync.dma_start(ch["hx8"][:, 1, :], ins["x8_all"][t, p])
            step(ch, which, tau, ch["xs"][t3 % 2])
            if ci % 2 == 1:
                newk = ci // 2
                while pend and pend[0][0] < newk or (pend and pend[0][1] < t):
                    pk, pt = pend.pop(0)
                    tail(pk, pt in (T_OBS - 1, T_ALL - 1))
                pend.append((newk, t))
    while pend:
        pk, pt = pend.pop(0)
        tail(pk, pt in (T_OBS - 1, T_ALL - 1))
    for ci in range(N_CHAINS):
        nc.sync.dma_start(outs["hT_pre"][ci], chains[ci]["h"])
    state.release()
    psump.release()


_CACHED = {}


def _get_program():
    if "nc" in _CACHED:
        return _CACHED["nc"]
    nc = bacc.Bacc("TRN2", target_bir_lowering=False, debug=False,
                   enable_asserts=False, num_devices=N_CORES)
    in_specs = {
        "x_obs": ((N_CHUNK_OBS, N_PASS, 128, N), F16),
        "x_pre": ((N_CHUNK_PRE, N_PASS, 128, N), F16),
        "x8_all": ((T_ALL, N_PASS, 128, N), F8),
        "hT0": ((N_PASS, 128, N), F16),
        "hT0_8": ((N_PASS, 128, N), F8),
        "cT0": ((N_PASS, 128, N), F16),
        "cT0_pre": ((N_PASS, 128, N), F16),
        "w_dr_obs": ((128, 3, 2, 128), F8),
        "w_dr_pre": ((128, 3, 2, 128), F8),
        "w_gx_obs": ((XPACK, 128, 128), F16),
        "w_gx_pre": ((XPACK, 128, 128), F16),
        "w_gh_obs": ((128, 128), F16),
        "w_gh_pre": ((128, 128), F16),
    }
    ins = {
        k: nc.dram_tensor(k, list(s), dt, kind="ExternalInput").ap()
        for k, (s, dt) in in_specs.items()
    }
    outs = {
        k: nc.dram_tensor(k, [N_PASS, 128, N], F16, kind="ExternalOutput").ap()
        for k in ("hT_obs", "hT_pre")
    }
    with tile.TileContext(nc) as tc:
        _build_kernel(tc, outs, ins)
    nc.compile()
    _CACHED["nc"] = nc
    return nc


def run(inputs, trace=False, trace_kwargs=None):
    """Run the kernel on 8 cores; returns ((c_out, x_out), BassKernelResults)."""
    nc = _get_program()
    g = lambda k: np.asarray(inputs[k], np.float32)
    wo = _make_weights(g("W_in"), g("b_in"), g("W_ih_obs"),
                       g("W_hh_obs"), g("b_ih_obs"), g("b_hh_obs"))
    wp = _make_weights(g("W_in"), g("b_in"), g("W_ih_pre"),
                       g("W_hh_pre"), g("b_ih_pre"), g("b_hh_pre"))
    weights = {k + "_obs": v for k, v in wo.items()}
    weights.update({k + "_pre": v for k, v in wp.items()})
    in_maps = [
        _prep_core_inputs(inputs, c * B_C, (c + 1) * B_C, weights)
        for c in range(N_CORES)
    ]
    res = bass_utils.run_bass_kernel_spmd(
        nc, in_maps, core_ids=list(range(N_CORES)), trace=trace,
        **(trace_kwargs or {}))
    hT_obs = np.concatenate(
        [_unshuffle_state(res.results[c]["hT_obs"]) for c in range(N_CORES)],
        axis=1)
    hT_pre = np.concatenate(
        [_unshuffle_state(res.results[c]["hT_pre"]) for c in range(N_CORES)],
        axis=1)
    c_out = hT_obs.reshape(B, H).astype(np.float32)
    x_out = hT_pre.reshape(B, H).astype(np.float32)
    return (c_out, x_out), res


def kernel(**inputs):
    (c_out, x_out), _ = run(inputs)
    return c_out, x_out


# revision 12
# speedup vs baseline: 1.0484x; 1.0082x over previous
"""TRN2 Bass kernel for nn_Encoder (two-phase LSTM over huge batch).

Self-contained: takes the FULL unsharded inputs, shards the batch across
8 NeuronCores (pure data parallel), runs a Bass/Tile kernel per core via
run_bass_kernel_spmd, and reassembles the full outputs.

Device layout (per core, batch B_c = 65536):
  - batch split into 8 passes of 16*512; slice s=0..15 covers 512 columns
    of a pass; SBUF partition p = 8*s + r  <->  (slice s, feature r).
  - gates land in PSUM [128, 4, 512] banks [F, I, O, G] (pytorch gate
    order in the weight rows is i, f, g, o).
  - F/I/O banks: ONE fp8e4m3 DoubleRow matmul each per step: lhsT
    [128, 2, 128], rhs [128, 2, 512] where plane0 = h8 (fp8 copy of the
    hidden state) against block-diagonal W_hh8, plane1 = the x8 tile
    carrying split-precision input rows (x0hi, x0lo, x1hi, x1lo) and two
    ones rows weighted by the split bias (b_hi, b_lo).  Splitting data
    and bias across row pairs cancels their fp8 quantization error; the
    remaining error (fp8 W_hh/W_x weights + fp8 h) measures ~7e-3 l2 on
    the full model - well inside the 2e-2 budget.
  - G bank (tanh gate, most error-sensitive) stays fp16: two matmuls
    (x-part from a 3-step-packed fp16 x-tile with a ones/bias row, then
    h-part) as in the all-fp16 version.
  - This cuts PE work from 8 to 5 matmuls per step-pass; the PE was the
    baseline bottleneck (99% busy at the ~1.1GHz effective clock).
  - ACT: sigmoid over F/I/O in one instr, tanh over G, tanh(c).
  - DVE (fp16): u=F*c, v=I*G, c=u+v, h=O*tanh(c), plus an fp16->fp8
    copy of h into the DoubleRow rhs plane0.
  - 8 passes run as 8 interleaved chains, steps emitted round-robin so
    PSUM slots rotate across chains.
"""

import os
import sys

for _p in ("/opt/trn_rl_repo", "/root/.axon_site/_ro/trn_rl_repo"):
    if os.path.isdir(_p) and _p not in sys.path:
        sys.path.insert(0, _p)
        break

import numpy as np

import concourse.bacc as bacc
import concourse.mybir as mybir
import concourse.tile as tile
from concourse import bass_utils
import bass_rust
from concourse.alu_op_type import AluOpType

F32 = mybir.dt.float32
F16 = mybir.dt.float16
F8 = mybir.dt.float8e4
NP_F8 = bass_utils.ml_dtypes.float8_e4m3
AF = mybir.ActivationFunctionType
MPM = bass_rust.MatmulPerfMode

B = 524288
N_CORES = 8
B_C = B // N_CORES
N = 512
SLICES = 16
PASS = SLICES * N
N_PASS = B_C // PASS
T_OBS, T_PRE, IN, H = 8, 12, 2, 8
T_ALL = T_OBS + T_PRE
XPACK = 3
N_CHUNK_OBS = (T_OBS + XPACK - 1) // XPACK
N_CHUNK_PRE = (T_PRE + XPACK - 1) // XPACK
N_CHAINS = 8
# PSUM bank order: F, I, O, G (sigmoid banks contiguous, tanh last);
# pytorch gate order in the weight rows is i, f, g, o.
BANK_GATE = [1, 0, 3, 2]


# ---------------------------------------------------------------- host prep

def _q8(a):
    return a.astype(NP_F8).astype(np.float32)


def _make_weights(W_in, b_in, W_ih, W_hh, b_ih, b_hh):
    """Weights for one LSTM phase.

    Returns dict with:
      w_dr  [128, 3, 2, 128] fp8: DoubleRow lhsT per F/I/O bank
             (plane0 = block-diag W_hh8, plane1 = x-split + bias rows)
      w_gx  [XPACK, 128, 128] fp16: G-bank x lhsT (bias on ones row)
      w_gh  [128, 128] fp16: G-bank h lhsT
    """
    Wx = (W_ih @ W_in).astype(np.float32)
    bias = (W_ih @ b_in + b_ih + b_hh).astype(np.float32)
    Wx8 = _q8(Wx)
    Wh8 = _q8(W_hh.astype(np.float32))
    b_hi = _q8(bias)
    b_lo = _q8(bias - b_hi)

    w_dr = np.zeros((128, 3, 2, 128), np.float32)
    for b in range(3):
        g = BANK_GATE[b]
        for s in range(16):
            for r in range(H):
                col = 8 * s + r
                w_dr[8 * s: 8 * s + H, b, 0, col] = Wh8[g * H + r, :]
                w_dr[8 * s + 0, b, 1, col] = Wx8[g * H + r, 0]
                w_dr[8 * s + 1, b, 1, col] = Wx8[g * H + r, 0]
                w_dr[8 * s + 2, b, 1, col] = Wx8[g * H + r, 1]
                w_dr[8 * s + 3, b, 1, col] = Wx8[g * H + r, 1]
                w_dr[8 * s + 4, b, 1, col] = b_hi[g * H + r]
                w_dr[8 * s + 5, b, 1, col] = b_lo[g * H + r]

    g = BANK_GATE[3]
    # sigma-trick: bank G holds 2*(pre-activation) so tanh(g) = 2*sig(2g)-1
    # comes out of the same all-sigmoid ACT instruction as F/I/O.
    w_gx = np.zeros((XPACK, 128, 128), np.float32)
    w_gh = np.zeros((128, 128), np.float32)
    for s in range(16):
        for r in range(H):
            col = 8 * s + r
            for tau in range(XPACK):
                for k in range(IN):
                    w_gx[tau, 8 * s + 2 * tau + k, col] = 2.0 * Wx[g * H + r, k]
                w_gx[tau, 8 * s + 6, col] = 2.0 * bias[g * H + r]
            w_gh[8 * s: 8 * s + H, col] = 2.0 * W_hh[g * H + r, :]
    return {
        "w_dr": w_dr.astype(NP_F8),
        "w_gx": w_gx.astype(np.float16),
        "w_gh": w_gh.astype(np.float16),
    }


def _shuffle_state(aT, dtype=np.float16):
    """[8, B_c] -> [N_PASS, 128, N] device layout (p, 8s+r, n)."""
    return np.ascontiguousarray(
        aT.reshape(H, N_PASS, SLICES, N).transpose(1, 2, 0, 3).reshape(
            N_PASS, 128, N).astype(dtype))


def _unshuffle_state(dev):
    """[N_PASS, 128, N] -> [8, B_c]."""
    return dev.reshape(N_PASS, SLICES, H, N).transpose(2, 0, 1, 3).reshape(
        H, B_C)


def _pack_x16(x):
    """[T, 2, B_c] -> [n_chunk, N_PASS, 128, N] fp16 (G bank):
    row 2*tau+k = x[t0+tau][k], row 6 = ones."""
    T = x.shape[0]
    n_chunk = (T + XPACK - 1) // XPACK
    out = np.zeros((n_chunk, N_PASS, SLICES, 8, N), np.float32)
    out[:, :, :, 6, :] = 1.0
    for tau in range(XPACK):
        for k in range(IN):
            for t3 in range(n_chunk):
                t = t3 * XPACK + tau
                if t < T:
                    out[t3, :, :, 2 * tau + k, :] = x[t, k].reshape(
                        N_PASS, SLICES, N)
    return np.ascontiguousarray(
        out.reshape(n_chunk, N_PASS, 128, N).astype(np.float16))


def _pack_x8(x):
    """[T, 2, B_c] -> [T, N_PASS, 128, N] fp8 split-precision rows:
    rows per slice: x0hi, x0lo, x1hi, x1lo, 1, 1, 0, 0."""
    T = x.shape[0]
    out = np.zeros((T, N_PASS, SLICES, 8, N), np.float32)
    for k in range(IN):
        v = x[:, k, :].reshape(T, N_PASS, SLICES, N)
        hi = _q8(v)
        lo = v - hi
        out[:, :, :, 2 * k, :] = hi
        out[:, :, :, 2 * k + 1, :] = lo
    out[:, :, :, 4, :] = 1.0
    out[:, :, :, 5, :] = 1.0
    return np.ascontiguousarray(
        out.reshape(T, N_PASS, 128, N).astype(NP_F8))


def _prep_core_inputs(inputs, lo, hi, weights):
    g = lambda k: np.asarray(inputs[k], np.float32)
    xo = np.ascontiguousarray(g("obs_traj_rel")[:, lo:hi, :].transpose(0, 2, 1))
    xp = np.ascontiguousarray(g("pre_traj_rel")[:, lo:hi, :].transpose(0, 2, 1))
    d = {}
    d["x_obs"] = _pack_x16(xo)
    d["x_pre"] = _pack_x16(xp)
    d["x8_all"] = np.concatenate([_pack_x8(xo), _pack_x8(xp)], axis=0)
    hT0 = np.ascontiguousarray(g("h0")[lo:hi].T)
    d["hT0"] = _shuffle_state(hT0)
    d["hT0_8"] = _shuffle_state(hT0, NP_F8)
    d["cT0"] = _shuffle_state(np.ascontiguousarray(g("c0")[lo:hi].T))
    d["cT0_pre"] = _shuffle_state(np.ascontiguousarray(g("c0_pre")[lo:hi].T))
    d.update(weights)
    return d


# ------------------------------------------------------------- device build

def _build_kernel(tc, outs, ins):
    nc = tc.nc
    state = tc.alloc_tile_pool(name="state", bufs=1)
    psump = tc.alloc_tile_pool(name="psum", bufs=2, space="PSUM")

    wsb = {}
    for key in ("w_dr_obs", "w_dr_pre"):
        w = state.tile([128, 3, 2, 128], F8, name=key + "_sb", tag=key)
        nc.sync.dma_start(w, ins[key])
        wsb[key] = w
    for key in ("w_gx_obs", "w_gx_pre"):
        w = state.tile([128, XPACK, 128], F16, name=key + "_sb", tag=key)
        nc.sync.dma_start(w, ins[key].rearrange("t p m -> p t m"))
        wsb[key] = w
    for key in ("w_gh_obs", "w_gh_pre"):
        w = state.tile([128, 128], F16, name=key + "_sb", tag=key)
        nc.sync.dma_start(w, ins[key])
        wsb[key] = w

    cpair, tcpair = [], []
    for k in range(N_CHAINS // 2):
        cpair.append(state.tile([128, 2, N], F16, name=f"cp_{k}", tag=f"cp_{k}"))
        tcpair.append(state.tile([128, 2, N], F16, name=f"tcp_{k}",
                                 tag=f"tcp_{k}"))
    chains = []
    for ci in range(N_CHAINS):
        ch = {}
        for nm in ("h", "u", "v", "g2"):
            ch[nm] = state.tile([128, N], F16, name=f"{nm}_{ci}",
                                tag=f"{nm}_{ci}")
        ch["c"] = cpair[ci // 2][:, ci % 2]
        ch["tc"] = tcpair[ci // 2][:, ci % 2]
        ch["hx8"] = state.tile([128, 2, N], F8, name=f"hx8_{ci}",
                               tag=f"hx8_{ci}")
        ch["T"] = state.tile([128, 4, N], F16, name=f"T_{ci}", tag=f"T_{ci}")
        ch["xs"] = [
            state.tile([128, N], F16, name=f"x_{ci}_{xi}", tag=f"x_{ci}_{xi}")
            for xi in range(2)
        ]
        chains.append(ch)
    def step(ch, which, tau, xt):
        wdr = wsb[f"w_dr_{which}"]
        wgx, wgh = wsb[f"w_gx_{which}"], wsb[f"w_gh_{which}"]
        ps = psump.tile([128, 4, 512], F32, name="ps", tag="ps")
        for b in range(3):
            nc.tensor.matmul(ps[:, b, :N], wdr[:, b, :, :], ch["hx8"],
                             start=True, stop=True, perf_mode=MPM.DoubleRow)
        out = ps[:, 3, :N]
        nc.tensor.matmul(out, wgx[:, tau, :], xt, start=True, stop=False)
        nc.tensor.matmul(out, wgh, ch["hx8"][:, 0, :], start=False, stop=True)
        T = ch["T"]
        nc.scalar.activation(T, ps[:, :, :N], AF.Sigmoid)
        nc.vector.tensor_scalar(ch["g2"], T[:, 3, :], 2.0, -1.0,
                                AluOpType.mult, AluOpType.add)   # G = 2s-1
        nc.vector.tensor_mul(ch["u"], T[:, 0, :], ch["c"])       # u = F*c
        nc.vector.tensor_mul(ch["v"], T[:, 1, :], ch["g2"])      # v = I*G
        nc.vector.tensor_add(ch["c"], ch["u"], ch["v"])          # c = u+v

    def tail(k, want_h16):
        nc.scalar.activation(tcpair[k], cpair[k], AF.Tanh)       # tanh(c) x2
        for ci in (2 * k, 2 * k + 1):
            ch = chains[ci]
            nc.vector.tensor_mul(ch["hx8"][:, 0, :], ch["T"][:, 2, :],
                                 ch["tc"])                       # h8 = O*tc
            if want_h16:
                nc.vector.tensor_mul(ch["h"], ch["T"][:, 2, :], ch["tc"])
    assert N_PASS == N_CHAINS
    pend = []
    for t in range(T_ALL):
        if t < T_OBS:
            which, tt = "obs", t
        else:
            which, tt = "pre", t - T_OBS
        t3, tau = divmod(tt, XPACK)
        if t == T_OBS:
            while pend:
                pk, pt = pend.pop(0)
                tail(pk, pt in (T_OBS - 1, T_ALL - 1))
        for ci in range(N_CHAINS):
            ch, p = chains[ci], ci
            if t == 0:
                nc.gpsimd.dma_start(ch["h"], ins["hT0"][p])
                nc.gpsimd.dma_start(ch["hx8"][:, 0, :], ins["hT0_8"][p])
                nc.gpsimd.dma_start(ch["c"], ins["cT0"][p])
            if t == T_OBS:
                nc.sync.dma_start(outs["hT_obs"][p], ch["h"])
                nc.gpsimd.dma_start(ch["c"], ins["cT0_pre"][p])
            if tau == 0:
                nc.gpsimd.dma_start(ch["xs"][t3 % 2], ins[f"x_{which}"][t3, p])
            nc.gpsimd.dma_start(ch["hx8"][:, 1, :], ins["x8_all"][t, p])
            step(ch, which, tau, ch["xs"][t3 % 2])
            if ci % 2 == 1:
                newk = ci // 2
                while pend and pend[0][0] < newk or (pend and pend[0][1] < t):
                    pk, pt = pend.pop(0)
                    tail(pk, pt in (T_OBS - 1, T_ALL - 1))
                pend.append((newk, t))
    while pend:
        pk, pt = pend.pop(0)
        tail(pk, pt in (T_OBS - 1, T_ALL - 1))
    for ci in range(N_CHAINS):
        nc.sync.dma_start(outs["hT_pre"][ci], chains[ci]["h"])
    state.release()
    psump.release()


_CACHED = {}


def _get_program():
    if "nc" in _CACHED:
        return _CACHED["nc"]
    nc = bacc.Bacc("TRN2", target_bir_lowering=False, debug=False,
                   enable_asserts=False, num_devices=N_CORES)
    in_specs = {
        "x_obs": ((N_CHUNK_OBS, N_PASS, 128, N), F16),
        "x_pre": ((N_CHUNK_PRE, N_PASS, 128, N), F16),
        "x8_all": ((T_ALL, N_PASS, 128, N), F8),
        "hT0": ((N_PASS, 128, N), F16),
        "hT0_8": ((N_PASS, 128, N), F8),
        "cT0": ((N_PASS, 128, N), F16),
        "cT0_pre": ((N_PASS, 128, N), F16),
        "w_dr_obs": ((128, 3, 2, 128), F8),
        "w_dr_pre": ((128, 3, 2, 128), F8),
        "w_gx_obs": ((XPACK, 128, 128), F16),
        "w_gx_pre": ((XPACK, 128, 128), F16),
        "w_gh_obs": ((128, 128), F16),
        "w_gh_pre": ((128, 128), F16),
    }
    ins = {
        k: nc.dram_tensor(k, list(s), dt, kind="ExternalInput").ap()
        for k, (s, dt) in in_specs.items()
    }
    outs = {
        k: nc.dram_tensor(k, [N_PASS, 128, N], F16, kind="ExternalOutput").ap()
        for k in ("hT_obs", "hT_pre")
    }
    with tile.TileContext(nc) as tc:
        _build_kernel(tc, outs, ins)
    nc.compile()
    _CACHED["nc"] = nc
    return nc


def run(inputs, trace=False, trace_kwargs=None):
    """Run the kernel on 8 cores; returns ((c_out, x_out), BassKernelResults)."""
    nc = _get_program()
    g = lambda k: np.asarray(inputs[k], np.float32)
    wo = _make_weights(g("W_in"), g("b_in"), g("W_ih_obs"),
                       g("W_hh_obs"), g("b_ih_obs"), g("b_hh_obs"))
    wp = _make_weights(g("W_in"), g("b_in"), g("W_ih_pre"),
                       g("W_hh_pre"), g("b_ih_pre"), g("b_hh_pre"))
    weights = {k + "_obs": v for k, v in wo.items()}
    weights.update({k + "_pre": v for k, v in wp.items()})
    in_maps = [
        _prep_core_inputs(inputs, c * B_C, (c + 1) * B_C, weights)
        for c in range(N_CORES)
    ]
    res = bass_utils.run_bass_kernel_spmd(
        nc, in_maps, core_ids=list(range(N_CORES)), trace=trace,
        **(trace_kwargs or {}))
    hT_obs = np.concatenate(
        [_unshuffle_state(res.results[c]["hT_obs"]) for c in range(N_CORES)],
        axis=1)
    hT_pre = np.concatenate(
        [_unshuffle_state(res.results[c]["hT_pre"]) for c in range(N_CORES)],
        axis=1)
    c_out = hT_obs.reshape(B, H).astype(np.float32)
    x_out = hT_pre.reshape(B, H).astype(np.float32)
    return (c_out, x_out), res


def kernel(**inputs):
    (c_out, x_out), _ = run(inputs)
    return c_out, x_out


# revision 15
# speedup vs baseline: 1.0529x; 1.0043x over previous
"""TRN2 Bass kernel for nn_Encoder (two-phase LSTM over huge batch).

Self-contained: takes the FULL unsharded inputs, shards the batch across
8 NeuronCores (pure data parallel), runs a Bass/Tile kernel per core via
run_bass_kernel_spmd, and reassembles the full outputs.

Device layout (per core, batch B_c = 65536):
  - batch split into 8 passes of 16*512; slice s=0..15 covers 512 columns
    of a pass; SBUF partition p = 8*s + r  <->  (slice s, feature r).
  - gates land in PSUM [128, 4, 512] banks [F, I, O, G] (pytorch gate
    order in the weight rows is i, f, g, o).
  - F/I/O banks: ONE fp8e4m3 DoubleRow matmul each per step: lhsT
    [128, 2, 128], rhs [128, 2, 512] where plane0 = h8 (fp8 copy of the
    hidden state) against block-diagonal W_hh8, plane1 = the x8 tile
    carrying split-precision input rows (x0hi, x0lo, x1hi, x1lo) and two
    ones rows weighted by the split bias (b_hi, b_lo).  Splitting data
    and bias across row pairs cancels their fp8 quantization error; the
    remaining error (fp8 W_hh/W_x weights + fp8 h) measures ~7e-3 l2 on
    the full model - well inside the 2e-2 budget.
  - G bank (tanh gate, most error-sensitive) stays fp16: two matmuls
    (x-part from a 3-step-packed fp16 x-tile with a ones/bias row, then
    h-part) as in the all-fp16 version.
  - This cuts PE work from 8 to 5 matmuls per step-pass; the PE was the
    baseline bottleneck (99% busy at the ~1.1GHz effective clock).
  - ACT: sigmoid over F/I/O in one instr, tanh over G, tanh(c).
  - DVE (fp16): u=F*c, v=I*G, c=u+v, h=O*tanh(c), plus an fp16->fp8
    copy of h into the DoubleRow rhs plane0.
  - 8 passes run as 8 interleaved chains, steps emitted round-robin so
    PSUM slots rotate across chains.
"""

import os
import sys

for _p in ("/opt/trn_rl_repo", "/root/.axon_site/_ro/trn_rl_repo"):
    if os.path.isdir(_p) and _p not in sys.path:
        sys.path.insert(0, _p)
        break

import numpy as np

import concourse.bacc as bacc
import concourse.mybir as mybir
import concourse.tile as tile
from concourse import bass_utils
import bass_rust
from concourse.alu_op_type import AluOpType

F32 = mybir.dt.float32
F16 = mybir.dt.float16
F8 = mybir.dt.float8e4
NP_F8 = bass_utils.ml_dtypes.float8_e4m3
AF = mybir.ActivationFunctionType
MPM = bass_rust.MatmulPerfMode

B = 524288
N_CORES = 8
B_C = B // N_CORES
N = 512
SLICES = 16
PASS = SLICES * N
N_PASS = B_C // PASS
T_OBS, T_PRE, IN, H = 8, 12, 2, 8
T_ALL = T_OBS + T_PRE
XPACK = 3
N_CHUNK_OBS = (T_OBS + XPACK - 1) // XPACK
N_CHUNK_PRE = (T_PRE + XPACK - 1) // XPACK
N_CHAINS = 8
# PSUM bank order: F, I, O, G (sigmoid banks contiguous, tanh last);
# pytorch gate order in the weight rows is i, f, g, o.
BANK_GATE = [1, 0, 3, 2]


# ---------------------------------------------------------------- host prep

def _q8(a):
    return a.astype(NP_F8).astype(np.float32)


def _make_weights(W_in, b_in, W_ih, W_hh, b_ih, b_hh):
    """Weights for one LSTM phase.

    Returns dict with:
      w_dr  [128, 3, 2, 128] fp8: DoubleRow lhsT per F/I/O bank
             (plane0 = block-diag W_hh8, plane1 = x-split + bias rows)
      w_gx  [XPACK, 128, 128] fp16: G-bank x lhsT (bias on ones row)
      w_gh  [128, 128] fp16: G-bank h lhsT
    """
    Wx = (W_ih @ W_in).astype(np.float32)
    bias = (W_ih @ b_in + b_ih + b_hh).astype(np.float32)
    Wx8 = _q8(Wx)
    Wh8 = _q8(W_hh.astype(np.float32))
    b_hi = _q8(bias)
    b_lo = _q8(bias - b_hi)

    w_dr = np.zeros((128, 3, 2, 128), np.float32)
    for b in range(3):
        g = BANK_GATE[b]
        for s in range(16):
            for r in range(H):
                col = 8 * s + r
                w_dr[8 * s: 8 * s + H, b, 0, col] = Wh8[g * H + r, :]
                w_dr[8 * s + 0, b, 1, col] = Wx8[g * H + r, 0]
                w_dr[8 * s + 1, b, 1, col] = Wx8[g * H + r, 0]
                w_dr[8 * s + 2, b, 1, col] = Wx8[g * H + r, 1]
                w_dr[8 * s + 3, b, 1, col] = Wx8[g * H + r, 1]
                w_dr[8 * s + 4, b, 1, col] = b_hi[g * H + r]
                w_dr[8 * s + 5, b, 1, col] = b_lo[g * H + r]

    g = BANK_GATE[3]
    # sigma-trick: bank G holds 2*(pre-activation) so tanh(g) = 2*sig(2g)-1
    # comes out of the same all-sigmoid ACT instruction as F/I/O.
    w_gx = np.zeros((XPACK, 128, 128), np.float32)
    w_gh = np.zeros((128, 128), np.float32)
    for s in range(16):
        for r in range(H):
            col = 8 * s + r
            for tau in range(XPACK):
                for k in range(IN):
                    w_gx[tau, 8 * s + 2 * tau + k, col] = 2.0 * Wx[g * H + r, k]
                w_gx[tau, 8 * s + 6, col] = 2.0 * bias[g * H + r]
            w_gh[8 * s: 8 * s + H, col] = 2.0 * W_hh[g * H + r, :]
    return {
        "w_dr": w_dr.astype(NP_F8),
        "w_gx": w_gx.astype(np.float16),
        "w_gh": w_gh.astype(np.float16),
    }


def _shuffle_state(aT, dtype=np.float16):
    """[8, B_c] -> [N_PASS, 128, N] device layout (p, 8s+r, n)."""
    return np.ascontiguousarray(
        aT.reshape(H, N_PASS, SLICES, N).transpose(1, 2, 0, 3).reshape(
            N_PASS, 128, N).astype(dtype))


def _unshuffle_state(dev):
    """[N_PASS, 128, N] -> [8, B_c]."""
    return dev.reshape(N_PASS, SLICES, H, N).transpose(2, 0, 1, 3).reshape(
        H, B_C)


def _pack_x16(x):
    """[T, 2, B_c] -> [n_chunk, N_PASS, 128, N] fp16 (G bank):
    row 2*tau+k = x[t0+tau][k], row 6 = ones."""
    T = x.shape[0]
    n_chunk = (T + XPACK - 1) // XPACK
    out = np.zeros((n_chunk, N_PASS, SLICES, 8, N), np.float32)
    out[:, :, :, 6, :] = 1.0
    for tau in range(XPACK):
        for k in range(IN):
            for t3 in range(n_chunk):
                t = t3 * XPACK + tau
                if t < T:
                    out[t3, :, :, 2 * tau + k, :] = x[t, k].reshape(
                        N_PASS, SLICES, N)
    return np.ascontiguousarray(
        out.reshape(n_chunk, N_PASS, 128, N).astype(np.float16))


def _pack_x8(x):
    """[T, 2, B_c] -> [T, N_PASS, 128, N] fp8 split-precision rows:
    rows per slice: x0hi, x0lo, x1hi, x1lo, 1, 1, 0, 0."""
    T = x.shape[0]
    out = np.zeros((T, N_PASS, SLICES, 8, N), np.float32)
    for k in range(IN):
        v = x[:, k, :].reshape(T, N_PASS, SLICES, N)
        hi = _q8(v)
        lo = v - hi
        out[:, :, :, 2 * k, :] = hi
        out[:, :, :, 2 * k + 1, :] = lo
    out[:, :, :, 4, :] = 1.0
    out[:, :, :, 5, :] = 1.0
    return np.ascontiguousarray(
        out.reshape(T, N_PASS, 128, N).astype(NP_F8))


def _prep_core_inputs(inputs, lo, hi, weights):
    g = lambda k: np.asarray(inputs[k], np.float32)
    xo = np.ascontiguousarray(g("obs_traj_rel")[:, lo:hi, :].transpose(0, 2, 1))
    xp = np.ascontiguousarray(g("pre_traj_rel")[:, lo:hi, :].transpose(0, 2, 1))
    d = {}
    d["x_obs"] = _pack_x16(xo)
    d["x_pre"] = _pack_x16(xp)
    d["x8_all"] = np.concatenate([_pack_x8(xo), _pack_x8(xp)], axis=0)
    hT0 = np.ascontiguousarray(g("h0")[lo:hi].T)
    d["hT0"] = _shuffle_state(hT0)
    d["hT0_8"] = _shuffle_state(hT0, NP_F8)
    d["cT0"] = _shuffle_state(np.ascontiguousarray(g("c0")[lo:hi].T))
    d["cT0_pre"] = _shuffle_state(np.ascontiguousarray(g("c0_pre")[lo:hi].T))
    d.update(weights)
    return d


# ------------------------------------------------------------- device build

def _build_kernel(tc, outs, ins):
    nc = tc.nc
    state = tc.alloc_tile_pool(name="state", bufs=1)
    psump = tc.alloc_tile_pool(name="psum", bufs=2, space="PSUM")

    wsb = {}
    for key in ("w_dr_obs", "w_dr_pre"):
        w = state.tile([128, 3, 2, 128], F8, name=key + "_sb", tag=key)
        nc.sync.dma_start(w, ins[key])
        wsb[key] = w
    for key in ("w_gx_obs", "w_gx_pre"):
        w = state.tile([128, XPACK, 128], F16, name=key + "_sb", tag=key)
        nc.sync.dma_start(w, ins[key].rearrange("t p m -> p t m"))
        wsb[key] = w
    for key in ("w_gh_obs", "w_gh_pre"):
        w = state.tile([128, 128], F16, name=key + "_sb", tag=key)
        nc.sync.dma_start(w, ins[key])
        wsb[key] = w

    cpair, tcpair = [], []
    for k in range(N_CHAINS // 2):
        cpair.append(state.tile([128, 2, N], F16, name=f"cp_{k}", tag=f"cp_{k}"))
        tcpair.append(state.tile([128, 2, N], F16, name=f"tcp_{k}",
                                 tag=f"tcp_{k}"))
    chains = []
    for ci in range(N_CHAINS):
        ch = {}
        for nm in ("h", "u", "v", "g2"):
            ch[nm] = state.tile([128, N], F16, name=f"{nm}_{ci}",
                                tag=f"{nm}_{ci}")
        ch["c"] = cpair[ci // 2][:, ci % 2]
        ch["tc"] = tcpair[ci // 2][:, ci % 2]
        ch["hx8"] = state.tile([128, 2, N], F8, name=f"hx8_{ci}",
                               tag=f"hx8_{ci}")
        ch["T"] = state.tile([128, 4, N], F16, name=f"T_{ci}", tag=f"T_{ci}")
        ch["xs"] = [
            state.tile([128, N], F16, name=f"x_{ci}_{xi}", tag=f"x_{ci}_{xi}")
            for xi in range(2)
        ]
        chains.append(ch)
    def step(ch, which, tau, xt):
        wdr = wsb[f"w_dr_{which}"]
        wgx, wgh = wsb[f"w_gx_{which}"], wsb[f"w_gh_{which}"]
        ps = psump.tile([128, 4, 512], F32, name="ps", tag="ps")
        for b in range(3):
            nc.tensor.matmul(ps[:, b, :N], wdr[:, b, :, :], ch["hx8"],
                             start=True, stop=True, perf_mode=MPM.DoubleRow)
        out = ps[:, 3, :N]
        nc.tensor.matmul(out, wgx[:, tau, :], xt, start=True, stop=False)
        nc.tensor.matmul(out, wgh, ch["hx8"][:, 0, :], start=False, stop=True)
        T = ch["T"]
        nc.scalar.activation(T, ps[:, :, :N], AF.Sigmoid)
        nc.vector.tensor_scalar(ch["g2"], T[:, 3, :], 2.0, -1.0,
                                AluOpType.mult, AluOpType.add)   # G = 2s-1
        nc.vector.tensor_mul(ch["u"], T[:, 0, :], ch["c"])       # u = F*c
        nc.vector.tensor_mul(ch["v"], T[:, 1, :], ch["g2"])      # v = I*G
        nc.vector.tensor_add(ch["c"], ch["u"], ch["v"])          # c = u+v

    def tail(k, want_h16, want_h8=True):
        nc.scalar.activation(tcpair[k], cpair[k], AF.Tanh)       # tanh(c) x2
        for ci in (2 * k, 2 * k + 1):
            ch = chains[ci]
            if want_h16:
                nc.vector.tensor_mul(ch["h"], ch["T"][:, 2, :], ch["tc"])
            if want_h8:
                nc.vector.tensor_mul(ch["hx8"][:, 0, :], ch["T"][:, 2, :],
                                     ch["tc"])                   # h8 = O*tc
    assert N_PASS == N_CHAINS
    pend = []
    for t in range(T_ALL):
        if t < T_OBS:
            which, tt = "obs", t
        else:
            which, tt = "pre", t - T_OBS
        t3, tau = divmod(tt, XPACK)
        if t == T_OBS:
            while pend:
                pk, pt = pend.pop(0)
                tail(pk, pt in (T_OBS - 1, T_ALL - 1))
        for ci in range(N_CHAINS):
            ch, p = chains[ci], ci
            if t == 0:
                eng = (nc.gpsimd, nc.sync)[ci % 2]
                eng.dma_start(ch["h"], ins["hT0"][p])
                eng.dma_start(ch["hx8"][:, 0, :], ins["hT0_8"][p])
                eng.dma_start(ch["c"], ins["cT0"][p])
            if t == T_OBS:
                nc.sync.dma_start(outs["hT_obs"][p], ch["h"])
                nc.gpsimd.dma_start(ch["c"], ins["cT0_pre"][p])
            if tau == 0:
                nc.gpsimd.dma_start(ch["xs"][t3 % 2], ins[f"x_{which}"][t3, p])
            nc.gpsimd.dma_start(ch["hx8"][:, 1, :], ins["x8_all"][t, p])
            step(ch, which, tau, ch["xs"][t3 % 2])
            if ci % 2 == 1:
                newk = ci // 2
                while pend and pend[0][0] < newk or (pend and pend[0][1] < t):
                    pk, pt = pend.pop(0)
                    tail(pk, pt in (T_OBS - 1, T_ALL - 1))
                pend.append((newk, t))
    while pend:
        pk, pt = pend.pop(0)
        tail(pk, pt in (T_OBS - 1, T_ALL - 1), want_h8=pt != T_ALL - 1)
    for ci in range(N_CHAINS):
        eng = (nc.gpsimd, nc.sync)[ci % 2]
        eng.dma_start(outs["hT_pre"][ci], chains[ci]["h"])
    state.release()
    psump.release()


_CACHED = {}


def _get_program():
    if "nc" in _CACHED:
        return _CACHED["nc"]
    nc = bacc.Bacc("TRN2", target_bir_lowering=False, debug=False,
                   enable_asserts=False, num_devices=N_CORES)
    in_specs = {
        "x_obs": ((N_CHUNK_OBS, N_PASS, 128, N), F16),
        "x_pre": ((N_CHUNK_PRE, N_PASS, 128, N), F16),
        "x8_all": ((T_ALL, N_PASS, 128, N), F8),
        "hT0": ((N_PASS, 128, N), F16),
        "hT0_8": ((N_PASS, 128, N), F8),
        "cT0": ((N_PASS, 128, N), F16),
        "cT0_pre": ((N_PASS, 128, N), F16),
        "w_dr_obs": ((128, 3, 2, 128), F8),
        "w_dr_pre": ((128, 3, 2, 128), F8),
        "w_gx_obs": ((XPACK, 128, 128), F16),
        "w_gx_pre": ((XPACK, 128, 128), F16),
        "w_gh_obs": ((128, 128), F16),
        "w_gh_pre": ((128, 128), F16),
    }
    ins = {
        k: nc.dram_tensor(k, list(s), dt, kind="ExternalInput").ap()
        for k, (s, dt) in in_specs.items()
    }
    outs = {
        k: nc.dram_tensor(k, [N_PASS, 128, N], F16, kind="ExternalOutput").ap()
        for k in ("hT_obs", "hT_pre")
    }
    with tile.TileContext(nc) as tc:
        _build_kernel(tc, outs, ins)
    nc.compile()
    _CACHED["nc"] = nc
    return nc


def run(inputs, trace=False, trace_kwargs=None):
    """Run the kernel on 8 cores; returns ((c_out, x_out), BassKernelResults)."""
    nc = _get_program()
    g = lambda k: np.asarray(inputs[k], np.float32)
    wo = _make_weights(g("W_in"), g("b_in"), g("W_ih_obs"),
                       g("W_hh_obs"), g("b_ih_obs"), g("b_hh_obs"))
    wp = _make_weights(g("W_in"), g("b_in"), g("W_ih_pre"),
                       g("W_hh_pre"), g("b_ih_pre"), g("b_hh_pre"))
    weights = {k + "_obs": v for k, v in wo.items()}
    weights.update({k + "_pre": v for k, v in wp.items()})
    in_maps = [
        _prep_core_inputs(inputs, c * B_C, (c + 1) * B_C, weights)
        for c in range(N_CORES)
    ]
    res = bass_utils.run_bass_kernel_spmd(
        nc, in_maps, core_ids=list(range(N_CORES)), trace=trace,
        **(trace_kwargs or {}))
    hT_obs = np.concatenate(
        [_unshuffle_state(res.results[c]["hT_obs"]) for c in range(N_CORES)],
        axis=1)
    hT_pre = np.concatenate(
        [_unshuffle_state(res.results[c]["hT_pre"]) for c in range(N_CORES)],
        axis=1)
    c_out = hT_obs.reshape(B, H).astype(np.float32)
    x_out = hT_pre.reshape(B, H).astype(np.float32)
    return (c_out, x_out), res


def kernel(**inputs):
    (c_out, x_out), _ = run(inputs)
    return c_out, x_out


# revision 16
# speedup vs baseline: 1.0546x; 1.0016x over previous
"""TRN2 Bass kernel for nn_Encoder (two-phase LSTM over huge batch).

Self-contained: takes the FULL unsharded inputs, shards the batch across
8 NeuronCores (pure data parallel), runs a Bass/Tile kernel per core via
run_bass_kernel_spmd, and reassembles the full outputs.

Device layout (per core, batch B_c = 65536):
  - batch split into 8 passes of 16*512; slice s=0..15 covers 512 columns
    of a pass; SBUF partition p = 8*s + r  <->  (slice s, feature r).
  - gates land in PSUM [128, 4, 512] banks [F, I, O, G] (pytorch gate
    order in the weight rows is i, f, g, o).
  - F/I/O banks: ONE fp8e4m3 DoubleRow matmul each per step: lhsT
    [128, 2, 128], rhs [128, 2, 512] where plane0 = h8 (fp8 copy of the
    hidden state) against block-diagonal W_hh8, plane1 = the x8 tile
    carrying split-precision input rows (x0hi, x0lo, x1hi, x1lo) and two
    ones rows weighted by the split bias (b_hi, b_lo).  Splitting data
    and bias across row pairs cancels their fp8 quantization error; the
    remaining error (fp8 W_hh/W_x weights + fp8 h) measures ~7e-3 l2 on
    the full model - well inside the 2e-2 budget.
  - G bank (tanh gate, most error-sensitive) stays fp16: two matmuls
    (x-part from a 3-step-packed fp16 x-tile with a ones/bias row, then
    h-part) as in the all-fp16 version.
  - This cuts PE work from 8 to 5 matmuls per step-pass; the PE was the
    baseline bottleneck (99% busy at the ~1.1GHz effective clock).
  - ACT: sigmoid over F/I/O in one instr, tanh over G, tanh(c).
  - DVE (fp16): u=F*c, v=I*G, c=u+v, h=O*tanh(c), plus an fp16->fp8
    copy of h into the DoubleRow rhs plane0.
  - 8 passes run as 8 interleaved chains, steps emitted round-robin so
    PSUM slots rotate across chains.
"""

import os
import sys

for _p in ("/opt/trn_rl_repo", "/root/.axon_site/_ro/trn_rl_repo"):
    if os.path.isdir(_p) and _p not in sys.path:
        sys.path.insert(0, _p)
        break

import numpy as np

import concourse.bacc as bacc
import concourse.mybir as mybir
import concourse.tile as tile
from concourse import bass_utils
import bass_rust
from concourse.alu_op_type import AluOpType

F32 = mybir.dt.float32
F16 = mybir.dt.float16
F8 = mybir.dt.float8e4
NP_F8 = bass_utils.ml_dtypes.float8_e4m3
AF = mybir.ActivationFunctionType
MPM = bass_rust.MatmulPerfMode

B = 524288
N_CORES = 8
B_C = B // N_CORES
N = 512
SLICES = 16
PASS = SLICES * N
N_PASS = B_C // PASS
T_OBS, T_PRE, IN, H = 8, 12, 2, 8
T_ALL = T_OBS + T_PRE
XPACK = 3
N_CHUNK_OBS = (T_OBS + XPACK - 1) // XPACK
N_CHUNK_PRE = (T_PRE + XPACK - 1) // XPACK
N_CHAINS = 8
# PSUM bank order: F, I, O, G (sigmoid banks contiguous, tanh last);
# pytorch gate order in the weight rows is i, f, g, o.
BANK_GATE = [1, 0, 3, 2]


# ---------------------------------------------------------------- host prep

def _q8(a):
    return a.astype(NP_F8).astype(np.float32)


def _make_weights(W_in, b_in, W_ih, W_hh, b_ih, b_hh):
    """Weights for one LSTM phase.

    Returns dict with:
      w_dr  [128, 3, 2, 128] fp8: DoubleRow lhsT per F/I/O bank
             (plane0 = block-diag W_hh8, plane1 = x-split + bias rows)
      w_gx  [XPACK, 128, 128] fp16: G-bank x lhsT (bias on ones row)
      w_gh  [128, 128] fp16: G-bank h lhsT
    """
    Wx = (W_ih @ W_in).astype(np.float32)
    bias = (W_ih @ b_in + b_ih + b_hh).astype(np.float32)
    Wx8 = _q8(Wx)
    Wh8 = _q8(W_hh.astype(np.float32))
    b_hi = _q8(bias)
    b_lo = _q8(bias - b_hi)

    w_dr = np.zeros((128, 3, 2, 128), np.float32)
    for b in range(3):
        g = BANK_GATE[b]
        for s in range(16):
            for r in range(H):
                col = 8 * s + r
                w_dr[8 * s: 8 * s + H, b, 0, col] = Wh8[g * H + r, :]
                w_dr[8 * s + 0, b, 1, col] = Wx8[g * H + r, 0]
                w_dr[8 * s + 1, b, 1, col] = Wx8[g * H + r, 0]
                w_dr[8 * s + 2, b, 1, col] = Wx8[g * H + r, 1]
                w_dr[8 * s + 3, b, 1, col] = Wx8[g * H + r, 1]
                w_dr[8 * s + 4, b, 1, col] = b_hi[g * H + r]
                w_dr[8 * s + 5, b, 1, col] = b_lo[g * H + r]

    g = BANK_GATE[3]
    # sigma-trick: bank G holds 2*(pre-activation) so tanh(g) = 2*sig(2g)-1
    # comes out of the same all-sigmoid ACT instruction as F/I/O.
    w_gx = np.zeros((XPACK, 128, 128), np.float32)
    w_gh = np.zeros((128, 128), np.float32)
    for s in range(16):
        for r in range(H):
            col = 8 * s + r
            for tau in range(XPACK):
                for k in range(IN):
                    w_gx[tau, 8 * s + 2 * tau + k, col] = 2.0 * Wx[g * H + r, k]
                w_gx[tau, 8 * s + 6, col] = 2.0 * bias[g * H + r]
            w_gh[8 * s: 8 * s + H, col] = 2.0 * W_hh[g * H + r, :]
    return {
        "w_dr": w_dr.astype(NP_F8),
        "w_gx": w_gx.astype(np.float16),
        "w_gh": w_gh.astype(np.float16),
    }


def _shuffle_state(aT, dtype=np.float16):
    """[8, B_c] -> [N_PASS, 128, N] device layout (p, 8s+r, n)."""
    return np.ascontiguousarray(
        aT.reshape(H, N_PASS, SLICES, N).transpose(1, 2, 0, 3).reshape(
            N_PASS, 128, N).astype(dtype))


def _unshuffle_state(dev):
    """[N_PASS, 128, N] -> [8, B_c]."""
    return dev.reshape(N_PASS, SLICES, H, N).transpose(2, 0, 1, 3).reshape(
        H, B_C)


def _pack_x16(x):
    """[T, 2, B_c] -> [n_chunk, N_PASS, 128, N] fp16 (G bank):
    row 2*tau+k = x[t0+tau][k], row 6 = ones."""
    T = x.shape[0]
    n_chunk = (T + XPACK - 1) // XPACK
    out = np.zeros((n_chunk, N_PASS, SLICES, 8, N), np.float32)
    out[:, :, :, 6, :] = 1.0
    for tau in range(XPACK):
        for k in range(IN):
            for t3 in range(n_chunk):
                t = t3 * XPACK + tau
                if t < T:
                    out[t3, :, :, 2 * tau + k, :] = x[t, k].reshape(
                        N_PASS, SLICES, N)
    return np.ascontiguousarray(
        out.reshape(n_chunk, N_PASS, 128, N).astype(np.float16))


def _pack_x8(x):
    """[T, 2, B_c] -> [T, N_PASS, 128, N] fp8 split-precision rows:
    rows per slice: x0hi, x0lo, x1hi, x1lo, 1, 1, 0, 0."""
    T = x.shape[0]
    out = np.zeros((T, N_PASS, SLICES, 8, N), np.float32)
    for k in range(IN):
        v = x[:, k, :].reshape(T, N_PASS, SLICES, N)
        hi = _q8(v)
        lo = v - hi
        out[:, :, :, 2 * k, :] = hi
        out[:, :, :, 2 * k + 1, :] = lo
    out[:, :, :, 4, :] = 1.0
    out[:, :, :, 5, :] = 1.0
    return np.ascontiguousarray(
        out.reshape(T, N_PASS, 128, N).astype(NP_F8))


def _prep_core_inputs(inputs, lo, hi, weights):
    g = lambda k: np.asarray(inputs[k], np.float32)
    xo = np.ascontiguousarray(g("obs_traj_rel")[:, lo:hi, :].transpose(0, 2, 1))
    xp = np.ascontiguousarray(g("pre_traj_rel")[:, lo:hi, :].transpose(0, 2, 1))
    d = {}
    d["x_obs"] = _pack_x16(xo)
    d["x_pre"] = _pack_x16(xp)
    d["x8_all"] = np.concatenate([_pack_x8(xo), _pack_x8(xp)], axis=0)
    hT0 = np.ascontiguousarray(g("h0")[lo:hi].T)
    d["hT0"] = _shuffle_state(hT0)
    d["hT0_8"] = _shuffle_state(hT0, NP_F8)
    d["cT0"] = _shuffle_state(np.ascontiguousarray(g("c0")[lo:hi].T))
    d["cT0_pre"] = _shuffle_state(np.ascontiguousarray(g("c0_pre")[lo:hi].T))
    d.update(weights)
    return d


# ------------------------------------------------------------- device build

def _build_kernel(tc, outs, ins):
    nc = tc.nc
    state = tc.alloc_tile_pool(name="state", bufs=1)
    psump = tc.alloc_tile_pool(name="psum", bufs=2, space="PSUM")

    wsb = {}
    for key in ("w_dr_obs", "w_dr_pre"):
        w = state.tile([128, 3, 2, 128], F8, name=key + "_sb", tag=key)
        nc.sync.dma_start(w, ins[key])
        wsb[key] = w
    for key in ("w_gx_obs", "w_gx_pre"):
        w = state.tile([128, XPACK, 128], F16, name=key + "_sb", tag=key)
        nc.sync.dma_start(w, ins[key].rearrange("t p m -> p t m"))
        wsb[key] = w
    for key in ("w_gh_obs", "w_gh_pre"):
        w = state.tile([128, 128], F16, name=key + "_sb", tag=key)
        nc.sync.dma_start(w, ins[key])
        wsb[key] = w

    cpair, tcpair = [], []
    for k in range(N_CHAINS // 2):
        cpair.append(state.tile([128, 2, N], F16, name=f"cp_{k}", tag=f"cp_{k}"))
        tcpair.append(state.tile([128, 2, N], F16, name=f"tcp_{k}",
                                 tag=f"tcp_{k}"))
    chains = []
    for ci in range(N_CHAINS):
        ch = {}
        for nm in ("h", "u", "v", "g2"):
            ch[nm] = state.tile([128, N], F16, name=f"{nm}_{ci}",
                                tag=f"{nm}_{ci}")
        ch["c"] = cpair[ci // 2][:, ci % 2]
        ch["tc"] = tcpair[ci // 2][:, ci % 2]
        ch["hx8"] = state.tile([128, 2, N], F8, name=f"hx8_{ci}",
                               tag=f"hx8_{ci}")
        ch["T"] = state.tile([128, 4, N], F16, name=f"T_{ci}", tag=f"T_{ci}")
        ch["xs"] = [
            state.tile([128, N], F16, name=f"x_{ci}_{xi}", tag=f"x_{ci}_{xi}")
            for xi in range(2)
        ]
        chains.append(ch)
    def step(ch, which, tau, xt):
        wdr = wsb[f"w_dr_{which}"]
        wgx, wgh = wsb[f"w_gx_{which}"], wsb[f"w_gh_{which}"]
        ps = psump.tile([128, 4, 512], F32, name="ps", tag="ps")
        for b in range(3):
            nc.tensor.matmul(ps[:, b, :N], wdr[:, b, :, :], ch["hx8"],
                             start=True, stop=True, perf_mode=MPM.DoubleRow)
        out = ps[:, 3, :N]
        nc.tensor.matmul(out, wgx[:, tau, :], xt, start=True, stop=False)
        nc.tensor.matmul(out, wgh, ch["hx8"][:, 0, :], start=False, stop=True)
        T = ch["T"]
        nc.scalar.activation(T, ps[:, :, :N], AF.Sigmoid)
        nc.vector.tensor_scalar(ch["g2"], T[:, 3, :], 2.0, -1.0,
                                AluOpType.mult, AluOpType.add)   # G = 2s-1
        nc.vector.tensor_mul(ch["u"], T[:, 0, :], ch["c"])       # u = F*c
        nc.vector.tensor_mul(ch["v"], T[:, 1, :], ch["g2"])      # v = I*G
        nc.vector.tensor_add(ch["c"], ch["u"], ch["v"])          # c = u+v

    def tail(k, want_h16, want_h8=True):
        nc.scalar.activation(tcpair[k], cpair[k], AF.Tanh)       # tanh(c) x2
        for ci in (2 * k, 2 * k + 1):
            ch = chains[ci]
            if want_h16:
                nc.vector.tensor_mul(ch["h"], ch["T"][:, 2, :], ch["tc"])
            if want_h8:
                nc.vector.tensor_mul(ch["hx8"][:, 0, :], ch["T"][:, 2, :],
                                     ch["tc"])                   # h8 = O*tc
    assert N_PASS == N_CHAINS
    pend = []
    for t in range(T_ALL):
        if t < T_OBS:
            which, tt = "obs", t
        else:
            which, tt = "pre", t - T_OBS
        t3, tau = divmod(tt, XPACK)
        if t == T_OBS:
            while pend:
                pk, pt = pend.pop(0)
                tail(pk, pt in (T_OBS - 1, T_ALL - 1))
        for ci in range(N_CHAINS):
            ch, p = chains[ci], ci
            if t == 0:
                eng = (nc.gpsimd, nc.sync)[ci % 2]
                eng.dma_start(ch["h"], ins["hT0"][p])
                eng.dma_start(ch["hx8"][:, 0, :], ins["hT0_8"][p])
                eng.dma_start(ch["c"], ins["cT0"][p])
            if t == T_OBS:
                nc.sync.dma_start(outs["hT_obs"][p], ch["h"])
                nc.gpsimd.dma_start(ch["c"], ins["cT0_pre"][p])
            if tau == 0:
                nc.gpsimd.dma_start(ch["xs"][t3 % 2], ins[f"x_{which}"][t3, p])
            nc.gpsimd.dma_start(ch["hx8"][:, 1, :], ins["x8_all"][t, p])
            step(ch, which, tau, ch["xs"][t3 % 2])
            if ci % 2 == 1:
                newk = ci // 2
                while pend and (pend[0][0] < newk or pend[0][1] < t):
                    pk, pt = pend.pop(0)
                    tail(pk, pt in (T_OBS - 1, T_ALL - 1),
                         want_h8=pt != T_ALL - 1)
                pend.append((newk, t))
    while pend:
        pk, pt = pend.pop(0)
        tail(pk, pt in (T_OBS - 1, T_ALL - 1), want_h8=pt != T_ALL - 1)
    for ci in range(N_CHAINS):
        eng = (nc.gpsimd, nc.sync)[ci % 2]
        eng.dma_start(outs["hT_pre"][ci], chains[ci]["h"])
    state.release()
    psump.release()


_CACHED = {}


def _get_program():
    if "nc" in _CACHED:
        return _CACHED["nc"]
    nc = bacc.Bacc("TRN2", target_bir_lowering=False, debug=False,
                   enable_asserts=False, num_devices=N_CORES)
    in_specs = {
        "x_obs": ((N_CHUNK_OBS, N_PASS, 128, N), F16),
        "x_pre": ((N_CHUNK_PRE, N_PASS, 128, N), F16),
        "x8_all": ((T_ALL, N_PASS, 128, N), F8),
        "hT0": ((N_PASS, 128, N), F16),
        "hT0_8": ((N_PASS, 128, N), F8),
        "cT0": ((N_PASS, 128, N), F16),
        "cT0_pre": ((N_PASS, 128, N), F16),
        "w_dr_obs": ((128, 3, 2, 128), F8),
        "w_dr_pre": ((128, 3, 2, 128), F8),
        "w_gx_obs": ((XPACK, 128, 128), F16),
        "w_gx_pre": ((XPACK, 128, 128), F16),
        "w_gh_obs": ((128, 128), F16),
        "w_gh_pre": ((128, 128), F16),
    }
    ins = {
        k: nc.dram_tensor(k, list(s), dt, kind="ExternalInput").ap()
        for k, (s, dt) in in_specs.items()
    }
    outs = {
        k: nc.dram_tensor(k, [N_PASS, 128, N], F16, kind="ExternalOutput").ap()
        for k in ("hT_obs", "hT_pre")
    }
    with tile.TileContext(nc) as tc:
        _build_kernel(tc, outs, ins)
    nc.compile()
    _CACHED["nc"] = nc
    return nc


def run(inputs, trace=False, trace_kwargs=None):
    """Run the kernel on 8 cores; returns ((c_out, x_out), BassKernelResults)."""
    nc = _get_program()
    g = lambda k: np.asarray(inputs[k], np.float32)
    wo = _make_weights(g("W_in"), g("b_in"), g("W_ih_obs"),
                       g("W_hh_obs"), g("b_ih_obs"), g("b_hh_obs"))
    wp = _make_weights(g("W_in"), g("b_in"), g("W_ih_pre"),
                       g("W_hh_pre"), g("b_ih_pre"), g("b_hh_pre"))
    weights = {k + "_obs": v for k, v in wo.items()}
    weights.update({k + "_pre": v for k, v in wp.items()})
    in_maps = [
        _prep_core_inputs(inputs, c * B_C, (c + 1) * B_C, weights)
        for c in range(N_CORES)
    ]
    res = bass_utils.run_bass_kernel_spmd(
        nc, in_maps, core_ids=list(range(N_CORES)), trace=trace,
        **(trace_kwargs or {}))
    hT_obs = np.concatenate(
        [_unshuffle_state(res.results[c]["hT_obs"]) for c in range(N_CORES)],
        axis=1)
    hT_pre = np.concatenate(
        [_unshuffle_state(res.results[c]["hT_pre"]) for c in range(N_CORES)],
        axis=1)
    c_out = hT_obs.reshape(B, H).astype(np.float32)
    x_out = hT_pre.reshape(B, H).astype(np.float32)
    return (c_out, x_out), res


def kernel(**inputs):
    (c_out, x_out), _ = run(inputs)
    return c_out, x_out


# revision 17
# speedup vs baseline: 1.0549x; 1.0003x over previous
"""TRN2 Bass kernel for nn_Encoder (two-phase LSTM over huge batch).

Self-contained: takes the FULL unsharded inputs, shards the batch across
8 NeuronCores (pure data parallel), runs a Bass/Tile kernel per core via
run_bass_kernel_spmd, and reassembles the full outputs.

Per core (batch B_c = 65536 = 8 chains x 16 slices x 512 cols; SBUF
partition p = 8*s + r <-> (slice s, feature r)):

  PE - 5 matmuls per chain-step (was 8 all-fp16):
  - F/I/O banks: ONE fp8e4m3 DoubleRow matmul each: lhsT [128,2,128],
    rhs = (h8 plane, x8 plane). The x8 plane carries split-precision
    rows (x0hi, x0lo, x1hi, x1lo) plus two ones rows weighted by the
    split bias (b_hi, b_lo), which cancels the fp8 quantization of the
    input and bias; remaining error is fp8 weights + fp8 h.
  - G bank (tanh gate, most error-sensitive): fp16 weights, two matmuls
    (packed fp16 x-tile with ones/bias row, then W_hh x h8 - the PE
    accepts mixed fp16 lhsT x fp8 rhs exactly).

  ACT (the bottleneck, ~94% busy, 1 elem/lane/cycle spline engine):
  - all four gate banks through ONE sigmoid instr [128,4,512]: the G
    bank's weights are pre-doubled so tanh(g) = 2*sigmoid(2g)-1 ("sigma
    trick"); the affine fix runs on the DVE at 4x (tensor_scalar).
  - tanh(c) batched per chain-PAIR [128,2,512] (c tiles pair-shared),
    emitted with a 2-chain stagger so the in-order ACT queue never
    stalls on the DVE cell update.

  DVE: u=F*c, v=I*G', c=u+v (2x tensor_tensor), G'=2s-1 (4x
  tensor_scalar), h8=O*tanh(c) written directly as fp8 into the next
  DoubleRow rhs plane; fp16 h is materialized only on the two output
  steps.

  DMA: per-step x8 tiles and initial state issued from the GpSimd queue
  (25ns dispatch vs 565ns on SP) with double-buffered x8 planes.

Measured: 408us (baseline 577us), rel_err 8.2e-3 (tolerance 2e-2).
"""

import os
import sys

for _p in ("/opt/trn_rl_repo", "/root/.axon_site/_ro/trn_rl_repo"):
    if os.path.isdir(_p) and _p not in sys.path:
        sys.path.insert(0, _p)
        break

import numpy as np

import concourse.bacc as bacc
import concourse.mybir as mybir
import concourse.tile as tile
from concourse import bass_utils
import bass_rust
from concourse.alu_op_type import AluOpType

F32 = mybir.dt.float32
F16 = mybir.dt.float16
F8 = mybir.dt.float8e4
NP_F8 = bass_utils.ml_dtypes.float8_e4m3
AF = mybir.ActivationFunctionType
MPM = bass_rust.MatmulPerfMode

B = 524288
N_CORES = 8
B_C = B // N_CORES
N = 512
SLICES = 16
PASS = SLICES * N
N_PASS = B_C // PASS
T_OBS, T_PRE, IN, H = 8, 12, 2, 8
T_ALL = T_OBS + T_PRE
XPACK = 3
N_CHUNK_OBS = (T_OBS + XPACK - 1) // XPACK
N_CHUNK_PRE = (T_PRE + XPACK - 1) // XPACK
N_CHAINS = 8
# PSUM bank order: F, I, O, G (sigmoid banks contiguous, tanh last);
# pytorch gate order in the weight rows is i, f, g, o.
BANK_GATE = [1, 0, 3, 2]


# ---------------------------------------------------------------- host prep

def _q8(a):
    return a.astype(NP_F8).astype(np.float32)


def _make_weights(W_in, b_in, W_ih, W_hh, b_ih, b_hh):
    """Weights for one LSTM phase.

    Returns dict with:
      w_dr  [128, 3, 2, 128] fp8: DoubleRow lhsT per F/I/O bank
             (plane0 = block-diag W_hh8, plane1 = x-split + bias rows)
      w_gx  [XPACK, 128, 128] fp16: G-bank x lhsT (bias on ones row)
      w_gh  [128, 128] fp16: G-bank h lhsT
    """
    Wx = (W_ih @ W_in).astype(np.float32)
    bias = (W_ih @ b_in + b_ih + b_hh).astype(np.float32)
    Wx8 = _q8(Wx)
    Wh8 = _q8(W_hh.astype(np.float32))
    b_hi = _q8(bias)
    b_lo = _q8(bias - b_hi)

    w_dr = np.zeros((128, 3, 2, 128), np.float32)
    for b in range(3):
        g = BANK_GATE[b]
        for s in range(16):
            for r in range(H):
                col = 8 * s + r
                w_dr[8 * s: 8 * s + H, b, 0, col] = Wh8[g * H + r, :]
                w_dr[8 * s + 0, b, 1, col] = Wx8[g * H + r, 0]
                w_dr[8 * s + 1, b, 1, col] = Wx8[g * H + r, 0]
                w_dr[8 * s + 2, b, 1, col] = Wx8[g * H + r, 1]
                w_dr[8 * s + 3, b, 1, col] = Wx8[g * H + r, 1]
                w_dr[8 * s + 4, b, 1, col] = b_hi[g * H + r]
                w_dr[8 * s + 5, b, 1, col] = b_lo[g * H + r]

    g = BANK_GATE[3]
    # sigma-trick: bank G holds 2*(pre-activation) so tanh(g) = 2*sig(2g)-1
    # comes out of the same all-sigmoid ACT instruction as F/I/O.
    w_gx = np.zeros((XPACK, 128, 128), np.float32)
    w_gh = np.zeros((128, 128), np.float32)
    for s in range(16):
        for r in range(H):
            col = 8 * s + r
            for tau in range(XPACK):
                for k in range(IN):
                    w_gx[tau, 8 * s + 2 * tau + k, col] = 2.0 * Wx[g * H + r, k]
                w_gx[tau, 8 * s + 6, col] = 2.0 * bias[g * H + r]
            w_gh[8 * s: 8 * s + H, col] = 2.0 * W_hh[g * H + r, :]
    return {
        "w_dr": w_dr.astype(NP_F8),
        "w_gx": w_gx.astype(np.float16),
        "w_gh": w_gh.astype(np.float16),
    }


def _shuffle_state(aT, dtype=np.float16):
    """[8, B_c] -> [N_PASS, 128, N] device layout (p, 8s+r, n)."""
    return np.ascontiguousarray(
        aT.reshape(H, N_PASS, SLICES, N).transpose(1, 2, 0, 3).reshape(
            N_PASS, 128, N).astype(dtype))


def _unshuffle_state(dev):
    """[N_PASS, 128, N] -> [8, B_c]."""
    return dev.reshape(N_PASS, SLICES, H, N).transpose(2, 0, 1, 3).reshape(
        H, B_C)


def _pack_x16(x):
    """[T, 2, B_c] -> [n_chunk, N_PASS, 128, N] fp16 (G bank):
    row 2*tau+k = x[t0+tau][k], row 6 = ones."""
    T = x.shape[0]
    n_chunk = (T + XPACK - 1) // XPACK
    out = np.zeros((n_chunk, N_PASS, SLICES, 8, N), np.float32)
    out[:, :, :, 6, :] = 1.0
    for tau in range(XPACK):
        for k in range(IN):
            for t3 in range(n_chunk):
                t = t3 * XPACK + tau
                if t < T:
                    out[t3, :, :, 2 * tau + k, :] = x[t, k].reshape(
                        N_PASS, SLICES, N)
    return np.ascontiguousarray(
        out.reshape(n_chunk, N_PASS, 128, N).astype(np.float16))


def _pack_x8(x):
    """[T, 2, B_c] -> [T, N_PASS, 128, N] fp8 split-precision rows:
    rows per slice: x0hi, x0lo, x1hi, x1lo, 1, 1, 0, 0."""
    T = x.shape[0]
    out = np.zeros((T, N_PASS, SLICES, 8, N), np.float32)
    for k in range(IN):
        v = x[:, k, :].reshape(T, N_PASS, SLICES, N)
        hi = _q8(v)
        lo = v - hi
        out[:, :, :, 2 * k, :] = hi
        out[:, :, :, 2 * k + 1, :] = lo
    out[:, :, :, 4, :] = 1.0
    out[:, :, :, 5, :] = 1.0
    return np.ascontiguousarray(
        out.reshape(T, N_PASS, 128, N).astype(NP_F8))


def _prep_core_inputs(inputs, lo, hi, weights):
    g = lambda k: np.asarray(inputs[k], np.float32)
    xo = np.ascontiguousarray(g("obs_traj_rel")[:, lo:hi, :].transpose(0, 2, 1))
    xp = np.ascontiguousarray(g("pre_traj_rel")[:, lo:hi, :].transpose(0, 2, 1))
    d = {}
    d["x_obs"] = _pack_x16(xo)
    d["x_pre"] = _pack_x16(xp)
    d["x8_all"] = np.concatenate([_pack_x8(xo), _pack_x8(xp)], axis=0)
    hT0 = np.ascontiguousarray(g("h0")[lo:hi].T)
    d["hT0"] = _shuffle_state(hT0)
    d["hT0_8"] = _shuffle_state(hT0, NP_F8)
    d["cT0"] = _shuffle_state(np.ascontiguousarray(g("c0")[lo:hi].T))
    d["cT0_pre"] = _shuffle_state(np.ascontiguousarray(g("c0_pre")[lo:hi].T))
    d.update(weights)
    return d


# ------------------------------------------------------------- device build

def _build_kernel(tc, outs, ins):
    nc = tc.nc
    state = tc.alloc_tile_pool(name="state", bufs=1)
    psump = tc.alloc_tile_pool(name="psum", bufs=2, space="PSUM")

    wsb = {}
    for key in ("w_dr_obs", "w_dr_pre"):
        w = state.tile([128, 3, 2, 128], F8, name=key + "_sb", tag=key)
        nc.sync.dma_start(w, ins[key])
        wsb[key] = w
    for key in ("w_gx_obs", "w_gx_pre"):
        w = state.tile([128, XPACK, 128], F16, name=key + "_sb", tag=key)
        nc.sync.dma_start(w, ins[key].rearrange("t p m -> p t m"))
        wsb[key] = w
    for key in ("w_gh_obs", "w_gh_pre"):
        w = state.tile([128, 128], F16, name=key + "_sb", tag=key)
        nc.sync.dma_start(w, ins[key])
        wsb[key] = w

    cpair, tcpair = [], []
    for k in range(N_CHAINS // 2):
        cpair.append(state.tile([128, 2, N], F16, name=f"cp_{k}", tag=f"cp_{k}"))
        tcpair.append(state.tile([128, 2, N], F16, name=f"tcp_{k}",
                                 tag=f"tcp_{k}"))
    chains = []
    for ci in range(N_CHAINS):
        ch = {}
        for nm in ("h", "u", "v", "g2"):
            ch[nm] = state.tile([128, N], F16, name=f"{nm}_{ci}",
                                tag=f"{nm}_{ci}")
        ch["c"] = cpair[ci // 2][:, ci % 2]
        ch["tc"] = tcpair[ci // 2][:, ci % 2]
        ch["hx8"] = state.tile([128, 2, N], F8, name=f"hx8_{ci}",
                               tag=f"hx8_{ci}")
        ch["T"] = state.tile([128, 4, N], F16, name=f"T_{ci}", tag=f"T_{ci}")
        ch["xs"] = [
            state.tile([128, N], F16, name=f"x_{ci}_{xi}", tag=f"x_{ci}_{xi}")
            for xi in range(2)
        ]
        chains.append(ch)
    def step(ch, which, tau, xt):
        wdr = wsb[f"w_dr_{which}"]
        wgx, wgh = wsb[f"w_gx_{which}"], wsb[f"w_gh_{which}"]
        ps = psump.tile([128, 4, 512], F32, name="ps", tag="ps")
        for b in range(3):
            nc.tensor.matmul(ps[:, b, :N], wdr[:, b, :, :], ch["hx8"],
                             start=True, stop=True, perf_mode=MPM.DoubleRow)
        out = ps[:, 3, :N]
        nc.tensor.matmul(out, wgx[:, tau, :], xt, start=True, stop=False)
        nc.tensor.matmul(out, wgh, ch["hx8"][:, 0, :], start=False, stop=True)
        T = ch["T"]
        nc.scalar.activation(T, ps[:, :, :N], AF.Sigmoid)
        nc.vector.tensor_scalar(ch["g2"], T[:, 3, :], 2.0, -1.0,
                                AluOpType.mult, AluOpType.add)   # G = 2s-1
        nc.vector.tensor_mul(ch["u"], T[:, 0, :], ch["c"])       # u = F*c
        nc.vector.tensor_mul(ch["v"], T[:, 1, :], ch["g2"])      # v = I*G
        nc.vector.tensor_add(ch["c"], ch["u"], ch["v"])          # c = u+v

    def tail(k, want_h16, want_h8=True):
        nc.scalar.activation(tcpair[k], cpair[k], AF.Tanh)       # tanh(c) x2
        for ci in (2 * k, 2 * k + 1):
            ch = chains[ci]
            if want_h16:
                nc.vector.tensor_mul(ch["h"], ch["T"][:, 2, :], ch["tc"])
            if want_h8:
                nc.vector.tensor_mul(ch["hx8"][:, 0, :], ch["T"][:, 2, :],
                                     ch["tc"])                   # h8 = O*tc
    assert N_PASS == N_CHAINS
    pend = []
    for t in range(T_ALL):
        if t < T_OBS:
            which, tt = "obs", t
        else:
            which, tt = "pre", t - T_OBS
        t3, tau = divmod(tt, XPACK)
        if t == T_OBS:
            while pend:
                pk, pt = pend.pop(0)
                tail(pk, pt in (T_OBS - 1, T_ALL - 1))
        for ci in range(N_CHAINS):
            ch, p = chains[ci], ci
            if t == 0:
                eng = (nc.gpsimd, nc.sync)[ci % 2]
                eng.dma_start(ch["h"], ins["hT0"][p])
                eng.dma_start(ch["hx8"][:, 0, :], ins["hT0_8"][p])
                eng.dma_start(ch["c"], ins["cT0"][p])
            if t == T_OBS:
                nc.sync.dma_start(outs["hT_obs"][p], ch["h"])
                nc.gpsimd.dma_start(ch["c"], ins["cT0_pre"][p])
            if tau == 0:
                nc.gpsimd.dma_start(ch["xs"][t3 % 2], ins[f"x_{which}"][t3, p])
            nc.gpsimd.dma_start(ch["hx8"][:, 1, :], ins["x8_all"][t, p])
            step(ch, which, tau, ch["xs"][t3 % 2])
            if ci % 2 == 1:
                newk = ci // 2
                while pend and (pend[0][0] < newk or pend[0][1] < t):
                    pk, pt = pend.pop(0)
                    tail(pk, pt in (T_OBS - 1, T_ALL - 1),
                         want_h8=pt != T_ALL - 1)
                pend.append((newk, t))
    while pend:
        pk, pt = pend.pop(0)
        tail(pk, pt in (T_OBS - 1, T_ALL - 1), want_h8=pt != T_ALL - 1)
    for ci in range(N_CHAINS):
        eng = (nc.gpsimd, nc.sync)[ci % 2]
        eng.dma_start(outs["hT_pre"][ci], chains[ci]["h"])
    state.release()
    psump.release()


_CACHED = {}


def _get_program():
    if "nc" in _CACHED:
        return _CACHED["nc"]
    nc = bacc.Bacc("TRN2", target_bir_lowering=False, debug=False,
                   enable_asserts=False, num_devices=N_CORES)
    in_specs = {
        "x_obs": ((N_CHUNK_OBS, N_PASS, 128, N), F16),
        "x_pre": ((N_CHUNK_PRE, N_PASS, 128, N), F16),
        "x8_all": ((T_ALL, N_PASS, 128, N), F8),
        "hT0": ((N_PASS, 128, N), F16),
        "hT0_8": ((N_PASS, 128, N), F8),
        "cT0": ((N_PASS, 128, N), F16),
        "cT0_pre": ((N_PASS, 128, N), F16),
        "w_dr_obs": ((128, 3, 2, 128), F8),
        "w_dr_pre": ((128, 3, 2, 128), F8),
        "w_gx_obs": ((XPACK, 128, 128), F16),
        "w_gx_pre": ((XPACK, 128, 128), F16),
        "w_gh_obs": ((128, 128), F16),
        "w_gh_pre": ((128, 128), F16),
    }
    ins = {
        k: nc.dram_tensor(k, list(s), dt, kind="ExternalInput").ap()
        for k, (s, dt) in in_specs.items()
    }
    outs = {
        k: nc.dram_tensor(k, [N_PASS, 128, N], F16, kind="ExternalOutput").ap()
        for k in ("hT_obs", "hT_pre")
    }
    with tile.TileContext(nc) as tc:
        _build_kernel(tc, outs, ins)
    nc.compile()
    _CACHED["nc"] = nc
    return nc


def run(inputs, trace=False, trace_kwargs=None):
    """Run the kernel on 8 cores; returns ((c_out, x_out), BassKernelResults)."""
    nc = _get_program()
    g = lambda k: np.asarray(inputs[k], np.float32)
    wo = _make_weights(g("W_in"), g("b_in"), g("W_ih_obs"),
                       g("W_hh_obs"), g("b_ih_obs"), g("b_hh_obs"))
    wp = _make_weights(g("W_in"), g("b_in"), g("W_ih_pre"),
                       g("W_hh_pre"), g("b_ih_pre"), g("b_hh_pre"))
    weights = {k + "_obs": v for k, v in wo.items()}
    weights.update({k + "_pre": v for k, v in wp.items()})
    in_maps = [
        _prep_core_inputs(inputs, c * B_C, (c + 1) * B_C, weights)
        for c in range(N_CORES)
    ]
    res = bass_utils.run_bass_kernel_spmd(
        nc, in_maps, core_ids=list(range(N_CORES)), trace=trace,
        **(trace_kwargs or {}))
    hT_obs = np.concatenate(
        [_unshuffle_state(res.results[c]["hT_obs"]) for c in range(N_CORES)],
        axis=1)
    hT_pre = np.concatenate(
        [_unshuffle_state(res.results[c]["hT_pre"]) for c in range(N_CORES)],
        axis=1)
    c_out = hT_obs.reshape(B, H).astype(np.float32)
    x_out = hT_pre.reshape(B, H).astype(np.float32)
    return (c_out, x_out), res


def kernel(**inputs):
    (c_out, x_out), _ = run(inputs)
    return c_out, x_out
